# revision 9
# baseline (speedup 1.0000x reference)
"""DANetHead (dual attention head) Trainium2 kernel.

Strategy (8 NeuronCores): 2-way data parallel over batch B=2 (core groups
[0-3], [4-7]) x 4-way model parallel within each batch group.

Host->device traffic is minimized (it dominates the graded wall time):
  - x is channel-split: each core receives 512 of 2048 input channels
    (unreplicated). Stage-1 3x3 convs compute partial sums over those
    channels for ALL 512 output channels; an AllReduce over the batch
    group forms the true pre-activation, and bn_relu is applied locally.
  - Stage-1 conv weights: cores c and c+4 need identical blocks, so each
    receives half and an AllGather over pairs [[0,4],...] reconstructs.
  - Stage-2 conv weights + attention weights (identical on all 8 cores)
    are shipped 1/8th per core and AllGathered over all 8.

Matmuls run in bf16 (f32 PSUM accumulation) except the attention/CAM logits
which use f32 / hi-lo bf16 splitting to keep softmax inputs accurate.
"""

import os
from contextlib import ExitStack

import numpy as np
import ml_dtypes

import concourse.bass as bass
import concourse.tile as tile
import concourse.mybir as mybir
from concourse.bass import ds

dt = mybir.dt
F32 = dt.float32
BF16 = dt.bfloat16
AF = mybir.ActivationFunctionType
AX = mybir.AxisListType
ALU = mybir.AluOpType

P = 128
H = 60
HP = 62
NPIX = 3600          # 60*60
NPAD = 3720          # 60 zero + 3600 + 60 zero (padded full feature map)
MP = 3712            # 29*128, padded key/value pixel count
MCH = 29             # m chunks
WIN = 1020           # 17 rows * 60 query window
WINP = 1024          # padded window
CIN = 2048
CI = 512
CIC = 4              # 512 / 128
CQ = 64
CO = 40
CSH = 25.0           # softmax shift constant (max logit ~24.8)
GROUPS = [[0, 1, 2, 3], [4, 5, 6, 7]]
PAIRS = [[0, 4], [1, 5], [2, 6], [3, 7]]
WCH = 9 * CIC * P * CI // 8      # 294912  (1/8 of a stage-2 conv weight)
VCH = CIC * P * CI // 8          # 32768
OCH = 3 * CIC * P * CO // 8      # 7680
QCH = 2 * CIC * P * CQ // 8      # 8192
EPS = 1e-5

bf = ml_dtypes.bfloat16
_SKIP_CC = bool(int(os.environ.get("DANET_SKIP_CC", "0")))


# ---------------------------------------------------------------- builder ---

def build_nc(split=True, reps=1):
    nc = bass.Bass(num_devices=8)

    # ---- inputs (per-core contents differ; shapes identical) ----
    XP = nc.dram_tensor("XP", [CIC, P, NPIX], BF16, kind="ExternalInput")
    W0SH = nc.dram_tensor("W0SH", [2, P, 9, CI], BF16, kind="ExternalInput")
    W0CH = nc.dram_tensor("W0CH", [2, P, 9, CI], BF16, kind="ExternalInput")
    BN0S = nc.dram_tensor("BN0S", [P, 2, CIC], F32, kind="ExternalInput")
    BN0C = nc.dram_tensor("BN0C", [P, 2, CIC], F32, kind="ExternalInput")
    W1SP = nc.dram_tensor("W1SP", [WCH], BF16, kind="ExternalInput")
    W1CP = nc.dram_tensor("W1CP", [WCH], BF16, kind="ExternalInput")
    WVTP = nc.dram_tensor("WVTP", [VCH], BF16, kind="ExternalInput")
    W678P = nc.dram_tensor("W678P", [OCH], BF16, kind="ExternalInput")
    WQKP = nc.dram_tensor("WQKP", [QCH], F32, kind="ExternalInput")
    BQ = nc.dram_tensor("BQ", [CQ, 1], F32, kind="ExternalInput")
    BK = nc.dram_tensor("BK", [CQ, 1], F32, kind="ExternalInput")
    BV = nc.dram_tensor("BV", [P, CIC], F32, kind="ExternalInput")
    DKA = nc.dram_tensor("DKA", [2, MP], F32, kind="ExternalInput")
    DQA = nc.dram_tensor("DQA", [2, WINP], F32, kind="ExternalInput")
    QMASK = nc.dram_tensor("QMASK", [1, WINP], F32, kind="ExternalInput")
    GSA = nc.dram_tensor("GSA", [1, P], F32, kind="ExternalInput")
    GSC = nc.dram_tensor("GSC", [P, 1], F32, kind="ExternalInput")
    BN1S = nc.dram_tensor("BN1S", [P, 2, CIC], F32, kind="ExternalInput")
    BN1C = nc.dram_tensor("BN1C", [P, 2, CIC], F32, kind="ExternalInput")
    B6 = nc.dram_tensor("B6", [CO, 1], F32, kind="ExternalInput")
    B7 = nc.dram_tensor("B7", [CO, 1], F32, kind="ExternalInput")
    B8 = nc.dram_tensor("B8", [CO, 1], F32, kind="ExternalInput")
    OUT = nc.dram_tensor("OUT", [3, CO, 900], F32, kind="ExternalOutput")

    with tile.TileContext(nc) as tc:
        for _rep in range(reps):
            ctx = ExitStack()
            dram = ctx.enter_context(tc.tile_pool(name="dram", bufs=1, space="DRAM"))

            # window start within the padded full features: 900 * (core % 4)
            woff = (nc.sync.partition_id() % 4) * 900

            # gathered weights
            w0sg = dram.tile([2, 2, P, 9, CI], BF16, name="w0sg")
            w0cg = dram.tile([2, 2, P, 9, CI], BF16, name="w0cg")
            wqkg = dram.tile([2, CIC, P, CQ], F32, name="wqkg")
            wvg = dram.tile([CIC, P, CI], BF16, name="wvg")
            w1sg = dram.tile([9, CIC, P, CI], BF16, name="w1sg")
            w1cg = dram.tile([9, CIC, P, CI], BF16, name="w1cg")
            w678g = dram.tile([3, CIC, P, CO], BF16, name="w678g")

            # collectives cannot read IO tensors: stage inputs in DRAM tiles
            ALL8 = [[0, 1, 2, 3, 4, 5, 6, 7]]

            def ag(groups, src, shape, dtype, out_tile, name):
                st = dram.tile(shape, dtype, name=name)
                nc.sync.dma_start(st[:], src[:])
                nc.gpsimd.collective_compute(
                    "AllGather", ALU.bypass, replica_groups=groups,
                    ins=[st.opt()], outs=[out_tile.opt()],
                )

            ag(PAIRS, W0SH, [2, P, 9, CI], BF16, w0sg, "w0sh_st")
            ag(PAIRS, W0CH, [2, P, 9, CI], BF16, w0cg, "w0ch_st")
            ag(ALL8, WQKP, [QCH], F32, wqkg, "wqkp_st")
            ag(ALL8, WVTP, [VCH], BF16, wvg, "wvtp_st")
            ag(ALL8, W1SP, [WCH], BF16, w1sg, "w1sp_st")
            ag(ALL8, W1CP, [WCH], BF16, w1cg, "w1cp_st")
            ag(ALL8, W678P, [OCH], BF16, w678g, "w678p_st")

            # partial conv pre-activations and reduced versions
            p1 = dram.tile([CIC, P, NPIX], F32, name="p1")
            p2 = dram.tile([CIC, P, NPIX], F32, name="p2")
            p1r = dram.tile([CIC, P, NPIX], F32, name="p1r")
            p2r = dram.tile([CIC, P, NPIX], F32, name="p2r")
            f1g = dram.tile([CIC, P, NPAD], F32, name="f1g")
            f2g = dram.tile([CIC, P, NPAD], F32, name="f2g")
            cen_in = dram.tile([CIC, P, CI], F32, name="cen_in")
            cen_out = dram.tile([CIC, P, CI], F32, name="cen_out")

            # =========== stage 1: partial 3x3 convs (512 ins -> 512 outs) =======
            with ExitStack() as c1:
                sb1 = c1.enter_context(tc.tile_pool(name="sb1", bufs=1))
                fb1 = c1.enter_context(tc.tile_pool(name="fb1", bufs=2))
                pp1 = c1.enter_context(tc.tile_pool(name="pp1", bufs=8, space="PSUM"))

                zsb = sb1.tile([P, H], F32, name="zsb")
                nc.any.memset(zsb[:], 0.0)
                for fg_ in (f1g, f2g):
                    for cic in range(CIC):
                        nc.sync.dma_start(fg_[cic, :, 0:H], zsb[:])
                        nc.sync.dma_start(fg_[cic, :, NPAD - H: NPAD], zsb[:])

                xpad = sb1.tile([P, CIC, HP * HP], BF16, name="xpad")
                nc.any.memset(xpad[:], 0.0)
                for cic in range(CIC):
                    nc.sync.dma_start(
                        xpad[:, cic, :].rearrange(
                            "p (r c) -> p r c", c=HP)[:, 1:61, 1:61],
                        XP[cic].rearrange("p (r c) -> p r c", c=H),
                    )

                w0s_sb = [sb1.tile([P, 9, CI], BF16, name=f"w0s{i}")
                          for i in range(CIC)]
                w0c_sb = [sb1.tile([P, 9, CI], BF16, name=f"w0c{i}")
                          for i in range(CIC)]
                for cic in range(CIC):
                    nc.sync.dma_start(w0s_sb[cic][:], w0sg[cic // 2, cic % 2])
                    nc.sync.dma_start(w0c_sb[cic][:], w0cg[cic // 2, cic % 2])

                for wsb, pdst, prd in ((w0s_sb, p1, p1r), (w0c_sb, p2, p2r)):
                    for ot in range(CIC):
                        pts = [
                            pp1.tile([P, 480], F32, name="s1p", tag="s1p")
                            for _ in range(8)
                        ]
                        for cic in range(CIC):
                            xv = xpad[:, cic, :].rearrange("p (r c) -> p r c", c=HP)
                            for off in range(9):
                                ky, kx = off // 3, off % 3
                                start = cic == 0 and off == 0
                                stop = cic == CIC - 1 and off == 8
                                for t in range(8):
                                    rows = 8 if t < 7 else 4
                                    rhs = xv[:, ky + 8 * t: ky + 8 * t + rows,
                                             kx: kx + H]
                                    nc.tensor.matmul(
                                        pts[t][:, : rows * H],
                                        wsb[cic][:, off, ot * P: (ot + 1) * P],
                                        rhs, start=start, stop=stop,
                                    )
                        fbuf = fb1.tile([P, NPIX], F32, name="fbuf", tag="fbuf")
                        for t in range(8):
                            rows = 8 if t < 7 else 4
                            nc.scalar.activation(
                                fbuf[:, t * 480: t * 480 + rows * H],
                                pts[t][:, : rows * H], AF.Copy,
                            )
                        nc.sync.dma_start(pdst[ot], fbuf[:])
                    nc.gpsimd.collective_compute(
                        "AllReduce", ALU.add, replica_groups=GROUPS,
                        ins=[pdst.opt()], outs=[prd.opt()],
                    )

            # ====================== phase 2: bn_relu, windows, k, q, v ==========
            pers = ctx.enter_context(tc.tile_pool(name="pers", bufs=1))
            mid = ctx.enter_context(tc.tile_pool(name="mid", bufs=1))

            bn0s = pers.tile([P, 2, CIC], F32, name="bn0s")
            bn0c = pers.tile([P, 2, CIC], F32, name="bn0c")
            nc.sync.dma_start(bn0s[:], BN0S[:])
            nc.sync.dma_start(bn0c[:], BN0C[:])

            wqt = [pers.tile([P, CQ], F32, name=f"wqt{i}") for i in range(CIC)]
            wkt = [pers.tile([P, CQ], F32, name=f"wkt{i}") for i in range(CIC)]
            wvt = [pers.tile([P, CI], BF16, name=f"wvt{i}") for i in range(CIC)]
            for i in range(CIC):
                nc.sync.dma_start(wqt[i][:], wqkg[0, i])
                nc.sync.dma_start(wkt[i][:], wqkg[1, i])
                nc.sync.dma_start(wvt[i][:], wvg[i])
            bq = pers.tile([CQ, 1], F32, name="bq", padded_shape=[P, 1])
            bk = pers.tile([CQ, 1], F32, name="bk", padded_shape=[P, 1])
            bv = pers.tile([P, CIC], F32, name="bv")
            nc.sync.dma_start(bq[:], BQ[:])
            nc.sync.dma_start(bk[:], BK[:])
            nc.sync.dma_start(bv[:], BV[:])
            gsa = pers.tile([1, P], F32, name="gsa", padded_shape=[P, P])
            gsc = pers.tile([P, 1], F32, name="gsc")
            qmask = pers.tile([1, WINP], F32, name="qmask", padded_shape=[P, WINP])
            nc.sync.dma_start(gsa[:], GSA[:])
            nc.sync.dma_start(gsc[:], GSC[:])
            nc.sync.dma_start(qmask[:], QMASK[:])

            ka = mid.tile([P, MP], F32, name="ka")
            qa = mid.tile([P, WINP], F32, name="qa")
            kah = mid.tile([P, MP], BF16, name="kah")
            kal = mid.tile([P, MP], BF16, name="kal")
            qah = mid.tile([P, WINP], BF16, name="qah")
            qal = mid.tile([P, WINP], BF16, name="qal")
            nc.any.memset(ka[:], 0.0)
            nc.any.memset(qa[:], 0.0)
            nc.sync.dma_start(ka[64:66, :], DKA[:])
            nc.sync.dma_start(qa[64:66, :], DQA[:])

            f1win = [pers.tile([P, WINP], F32, name=f"f1win{i}") for i in range(CIC)]
            f2win = [pers.tile([P, WINP], F32, name=f"f2win{i}") for i in range(CIC)]
            vt = [pers.tile([P, MCH, P], BF16, name=f"vt{i}") for i in range(CIC)]

            with ExitStack() as c2:
                sb2 = c2.enter_context(tc.tile_pool(name="sb2", bufs=1))
                rp2 = c2.enter_context(tc.tile_pool(name="rp2", bufs=1))
                pk = c2.enter_context(tc.tile_pool(name="pk", bufs=8, space="PSUM"))

                vsp = c2.enter_context(tc.tile_pool(name="vsp", bufs=2))
                f1h = [sb2.tile([P, NPIX], BF16, name=f"f1h{i}") for i in range(CIC)]
                kps = [pk.tile([CQ, 450], F32, name="kp", tag="kp",
                               padded_shape=[P, 450]) for _ in range(8)]
                for cic in range(CIC):
                    r32 = rp2.tile([P, NPIX], F32, name="r32", tag="r32")
                    nc.sync.dma_start(r32[:], p1r[cic])
                    nc.scalar.activation(
                        r32[:], r32[:], AF.Relu,
                        bias=bn0s[:, 1, cic: cic + 1],
                        scale=bn0s[:, 0, cic: cic + 1],
                    )
                    nc.sync.dma_start(f1g[cic, :, H: H + NPIX], r32[:])
                    nc.vector.tensor_copy(f1h[cic][:], r32[:])
                    for nt in range(8):
                        nc.tensor.matmul(
                            kps[nt], wkt[cic][:], r32[:, nt * 450: (nt + 1) * 450],
                            start=cic == 0, stop=cic == CIC - 1,
                        )
                for nt in range(8):
                    nc.vector.tensor_scalar_add(
                        ka[0:CQ, nt * 450: (nt + 1) * 450], kps[nt], bk[:]
                    )

                # f2 = bn_relu(reduced partials), written to padded map
                for cic in range(CIC):
                    r32 = rp2.tile([P, NPIX], F32, name="r32", tag="r32")
                    nc.sync.dma_start(r32[:], p2r[cic])
                    nc.scalar.activation(
                        r32[:], r32[:], AF.Relu,
                        bias=bn0c[:, 1, cic: cic + 1],
                        scale=bn0c[:, 0, cic: cic + 1],
                    )
                    nc.sync.dma_start(f2g[cic, :, H: H + NPIX], r32[:])

                # per-core windows (rows 15s-1 .. 15s+16 incl. halo)
                for i in range(CIC):
                    nc.any.memset(f1win[i][:], 0.0)
                    nc.any.memset(f2win[i][:], 0.0)
                    nc.sync.dma_start(f1win[i][:, 0:WIN], f1g[i, :, ds(woff, WIN)])
                    nc.sync.dma_start(f2win[i][:, 0:WIN], f2g[i, :, ds(woff, WIN)])

                # q from the f32 window
                for hf in range(2):
                    qp = pk.tile([CQ, 512], F32, name="qp", tag="kp",
                                 padded_shape=[P, 512])
                    for cic in range(CIC):
                        nc.tensor.matmul(
                            qp, wqt[cic][:], f1win[cic][:, hf * 512: (hf + 1) * 512],
                            start=cic == 0, stop=cic == CIC - 1,
                        )
                    nc.vector.tensor_scalar_add(
                        qa[0:CQ, hf * 512: (hf + 1) * 512], qp, bq[:]
                    )

                # v = wv @ f1 (bf16), then transpose
                for cot in range(CIC):
                    vsb = vsp.tile([P, MP], BF16, name="vsb", tag="vsb")
                    nc.any.memset(vsb[:, NPIX:MP], 0.0)
                    for nt in range(8):
                        vp = pk.tile([P, 450], F32, name="vp", tag="kp")
                        for cic in range(CIC):
                            nc.tensor.matmul(
                                vp,
                                wvt[cic][:, cot * P: (cot + 1) * P],
                                f1h[cic][:, nt * 450: (nt + 1) * 450],
                                start=cic == 0, stop=cic == CIC - 1,
                            )
                        nc.vector.tensor_scalar_add(
                            vsb[:, nt * 450: (nt + 1) * 450], vp, bv[:, cot: cot + 1]
                        )
                    nc.sync.dma_start_transpose(vt[cot][:], vsb[:])

            # hi/lo packing for the energy matmul:
            #   mm1: lhsT=[kh(64); aug(2); 0] rhs=[qh(64); augq(2); 0]
            #   mm2: lhsT=[kl(64); kh(64)]    rhs=[qh(64); ql(64)]
            nc.vector.memset(kah[:], 0.0)
            nc.vector.memset(qah[:], 0.0)
            nc.vector.tensor_copy(kah[0:66, :], ka[0:66, :])
            nc.vector.tensor_sub(kal[0:64, :], ka[0:64, :], kah[0:64, :])
            nc.vector.tensor_copy(kal[64:128, :], kah[0:64, :])
            nc.vector.tensor_copy(qah[0:66, :], qa[0:66, :])
            nc.vector.tensor_sub(qal[64:128, :], qa[0:64, :], qah[0:64, :])
            nc.vector.tensor_copy(qal[0:64, :], qah[0:64, :])

            # ================= phase 4a: CAM gram matrix (overlaps AR) ===========
            xfwin = [pers.tile([P, WINP], BF16, name=f"xfwin{i}") for i in range(CIC)]
            cen_sb = [mid.tile([P, CI], F32, name=f"cen{i}") for i in range(CIC)]
            with ExitStack() as c4:
                sb4 = c4.enter_context(tc.tile_pool(name="sb4", bufs=1))
                pc = c4.enter_context(tc.tile_pool(name="pc", bufs=2, space="PSUM"))
                xfh = sb4.tile([P, CIC, WINP], BF16, name="xfh")
                xfl = sb4.tile([P, CIC, WINP], BF16, name="xfl")
                xth = sb4.tile([P, 8, CIC, P], BF16, name="xth")
                xtl = sb4.tile([P, 8, CIC, P], BF16, name="xtl")
                tmpf = sb4.tile([P, 900], F32, name="tmpf")
                for i in range(CIC):
                    nc.any.memset(xfwin[i][:], 0.0)
                    nc.vector.tensor_copy(xfwin[i][:, 0:WIN], f2win[i][:, 0:WIN])
                    nc.any.memset(xfh[:, i, 900:WINP], 0.0)
                    nc.any.memset(xfl[:, i, 900:WINP], 0.0)
                    # hi/lo split of my 900 pixels (window cols 60:960)
                    nc.vector.tensor_copy(xfh[:, i, 0:900], f2win[i][:, 60:960])
                    nc.vector.tensor_copy(tmpf[:], xfh[:, i, 0:900])
                    nc.vector.tensor_sub(xfl[:, i, 0:900], f2win[i][:, 60:960], tmpf[:])
                    nc.sync.dma_start_transpose(xth[:, :, i, :], xfh[:, i, :])
                    nc.sync.dma_start_transpose(xtl[:, :, i, :], xfl[:, i, :])
                for ct in range(CIC):
                    cp = pc.tile([P, CI], F32, name="cp", tag="cp")
                    n_mm = 0
                    for nch in range(8):
                        for lh, rh in ((xth, xth), (xth, xtl), (xtl, xth)):
                            nc.tensor.matmul(
                                cp, lh[:, nch, ct, :],
                                rh[:, nch, :, :].rearrange("p a b -> p (a b)"),
                                start=n_mm == 0, stop=n_mm == 23,
                            )
                            n_mm += 1
                    nc.scalar.activation(cen_sb[ct][:], cp[:], AF.Copy)
                    nc.sync.dma_start(cen_in[ct], cen_sb[ct][:])
                if not _SKIP_CC:
                    nc.gpsimd.collective_compute(
                        "AllReduce", ALU.add,
                        replica_groups=GROUPS,
                        ins=[cen_in.opt()], outs=[cen_out.opt()],
                    )
                else:
                    nc.sync.dma_start(cen_out[:], cen_in[:])

            # ======================= phase 3: position attention =================
            sa_win = [mid.tile([P, WINP], BF16, name=f"sawin{i}") for i in range(CIC)]
            with ExitStack() as c3:
                sb3 = c3.enter_context(tc.tile_pool(name="sb3", bufs=1))
                ap3 = c3.enter_context(tc.tile_pool(name="ap3", bufs=3))
                pe3 = c3.enter_context(tc.tile_pool(name="pe3", bufs=2, space="PSUM"))
                psa = c3.enter_context(tc.tile_pool(name="psa", bufs=4, space="PSUM"))
                psum3 = c3.enter_context(tc.tile_pool(name="psum3", bufs=2, space="PSUM"))

                ones = sb3.tile([P, 1], BF16, name="ones")
                nc.any.memset(ones[:], 1.0)
                nshift = sb3.tile([P, 1], F32, name="nshift")
                nc.any.memset(nshift[:], -CSH)
                for hf in range(2):
                    hsl = slice(hf * 512, (hf + 1) * 512)
                    saps = [
                        psa.tile([P, 512], F32, name="sap", tag="sap")
                        for _ in range(CIC)
                    ]
                    sums = psum3.tile([1, 512], F32, name="sums", tag="sums",
                                      padded_shape=[P, 512])
                    for mc in range(MCH):
                        ep = pe3.tile([P, 512], F32, name="ep", tag="ep")
                        nc.tensor.matmul(
                            ep, kah[:, mc * P: (mc + 1) * P], qah[:, hsl],
                            start=True, stop=False,
                        )
                        nc.tensor.matmul(
                            ep, kal[:, mc * P: (mc + 1) * P], qal[:, hsl],
                            start=False, stop=True,
                        )
                        at = ap3.tile([P, 512], BF16, name="at", tag="at")
                        nc.scalar.activation(at[:], ep[:], AF.Exp,
                                             bias=nshift[:], scale=1.0)
                        nc.tensor.matmul(
                            sums, ones[:], at[:], start=mc == 0, stop=mc == MCH - 1
                        )
                        for cot in range(CIC):
                            nc.tensor.matmul(
                                saps[cot], vt[cot][:, mc, :], at[:],
                                start=mc == 0, stop=mc == MCH - 1,
                            )
                    ssb = sb3.tile([1, 512], F32, name="ssb", tag="ssb",
                                   padded_shape=[P, 512])
                    nc.scalar.activation(ssb[:], sums[:], AF.Copy)
                    rec = sb3.tile([1, 512], F32, name="rec", tag="rec",
                                   padded_shape=[P, 512])
                    nc.vector.reciprocal(rec[:], ssb[:])
                    nc.vector.tensor_mul(rec[:], rec[:], qmask[:, hsl])
                    rbp = pe3.tile([P, 512], F32, name="rbp", tag="ep")
                    nc.tensor.matmul(rbp, gsa[:], rec[:], start=True, stop=True)
                    recb = sb3.tile([P, 512], F32, name="recb", tag="recb")
                    nc.scalar.activation(recb[:], rbp[:], AF.Copy)
                    for cot in range(CIC):
                        tmp3 = sb3.tile([P, 512], F32, name="tmp3", tag="tmp3")
                        nc.vector.tensor_mul(tmp3[:], saps[cot][:], recb[:])
                        nc.vector.tensor_add(
                            sa_win[cot][:, hsl], tmp3[:], f1win[cot][:, hsl]
                        )

            # =================== phase 4b: CAM softmax + attention ===============
            sc_win = [mid.tile([P, WINP], BF16, name=f"scwin{i}") for i in range(CIC)]
            with ExitStack() as c4b:
                sb4b = c4b.enter_context(tc.tile_pool(name="sb4b", bufs=1))
                pc2 = c4b.enter_context(tc.tile_pool(name="pc2", bufs=2, space="PSUM"))
                cattT = sb4b.tile([P, CIC, CIC, P], BF16, name="cattT")
                crec = sb4b.tile([P, CIC], F32, name="crec")
                for ct in range(CIC):
                    cg = cen_sb[ct]
                    nc.sync.dma_start(cg[:], cen_out[ct])
                    rmin = sb4b.tile([P, 1], F32, name="rmin", tag="rmin")
                    nc.vector.tensor_reduce(rmin[:], cg[:], axis=AX.X, op=ALU.min)
                    cat = sb4b.tile([P, CI], BF16, name="cat", tag="cat", bufs=2)
                    csum = sb4b.tile([P, 1], F32, name="csum", tag="csum", bufs=2)
                    nc.scalar.activation(
                        cat[:], cg[:], AF.Exp, bias=rmin[:], scale=-1.0,
                        accum_out=csum[:],
                    )
                    nc.vector.reciprocal(crec[:, ct: ct + 1], csum[:])
                    nc.vector.tensor_mul(crec[:, ct: ct + 1], crec[:, ct: ct + 1],
                                         gsc[:])
                    nc.sync.dma_start_transpose(cattT[:, :, ct, :], cat[:])
                for ct in range(CIC):
                    for hf in range(2):
                        hsl = slice(hf * 512, (hf + 1) * 512)
                        scp = pc2.tile([P, 512], F32, name="scp", tag="scp")
                        for dch in range(CIC):
                            nc.tensor.matmul(
                                scp, cattT[:, dch, ct, :], xfwin[dch][:, hsl],
                                start=dch == 0, stop=dch == CIC - 1,
                            )
                        tmp4 = sb4b.tile([P, 512], F32, name="tmp4", tag="tmp4")
                        nc.scalar.activation(tmp4[:], scp[:], AF.Copy,
                                             scale=crec[:, ct: ct + 1])
                        nc.vector.tensor_add(
                            sc_win[ct][:, hsl], tmp4[:], f2win[ct][:, hsl]
                        )

            # ============= phase 5: pads, stage-2 convs, output heads ============
            late = ctx.enter_context(tc.tile_pool(name="late", bufs=1))
            sa_pad = [late.tile([P, 17, HP], BF16, name=f"sapad{i}") for i in range(CIC)]
            sc_pad = [late.tile([P, 17, HP], BF16, name=f"scpad{i}") for i in range(CIC)]
            for i in range(CIC):
                nc.any.memset(sa_pad[i][:], 0.0)
                nc.any.memset(sc_pad[i][:], 0.0)
                nc.vector.tensor_copy(
                    sa_pad[i][:, :, 1:61],
                    sa_win[i][:, 0:WIN].rearrange("p (r c) -> p r c", c=H),
                )
                nc.vector.tensor_copy(
                    sc_pad[i][:, :, 1:61],
                    sc_win[i][:, 0:WIN].rearrange("p (r c) -> p r c", c=H),
                )

            sa_conv = [late.tile([P, 900], BF16, name=f"sacv{i}") for i in range(CIC)]
            sc_conv = [late.tile([P, 900], BF16, name=f"sccv{i}") for i in range(CIC)]
            fsum = [late.tile([P, 900], BF16, name=f"fsum{i}") for i in range(CIC)]

            with ExitStack() as c5:
                sb5 = c5.enter_context(tc.tile_pool(name="sb5", bufs=1))
                wp5 = c5.enter_context(tc.tile_pool(name="wp5", bufs=4))
                pp5 = c5.enter_context(tc.tile_pool(name="pp5", bufs=3, space="PSUM"))
                ph5 = c5.enter_context(tc.tile_pool(name="ph5", bufs=2, space="PSUM"))

                bn1 = sb5.tile([P, 2, 2, CIC], F32, name="bn1")
                nc.sync.dma_start(bn1[:, 0], BN1S[:])
                nc.sync.dma_start(bn1[:, 1], BN1C[:])

                for bi, (wsrc, pad, cv) in enumerate(
                    ((w1sg, sa_pad, sa_conv), (w1cg, sc_pad, sc_conv))
                ):
                    for cot in range(CIC):
                        cps = [
                            pp5.tile([P, 300], F32, name="cp5", tag="cp5")
                            for _ in range(3)
                        ]
                        for cic in range(CIC):
                            wt9 = wp5.tile([P, 9, P], BF16, name="w1t", tag="w1t")
                            nc.sync.dma_start(
                                wt9[:],
                                wsrc[:, cic, :, cot * P: (cot + 1) * P]
                                .rearrange("o p q -> p o q"))
                            for off in range(9):
                                ky, kx = off // 3, off % 3
                                start = cic == 0 and off == 0
                                stop = cic == CIC - 1 and off == 8
                                for rt in range(3):
                                    rhs = pad[cic][
                                        :, rt * 5 + ky: rt * 5 + ky + 5, kx: kx + H
                                    ]
                                    nc.tensor.matmul(
                                        cps[rt], wt9[:, off, :], rhs,
                                        start=start, stop=stop
                                    )
                        for rt in range(3):
                            nc.scalar.activation(
                                cv[cot][:, rt * 300: (rt + 1) * 300], cps[rt][:],
                                AF.Relu, bias=bn1[:, bi, 1, cot: cot + 1],
                                scale=bn1[:, bi, 0, cot: cot + 1],
                            )
                for i in range(CIC):
                    nc.vector.tensor_add(fsum[i][:], sa_conv[i][:], sc_conv[i][:])

                w6 = sb5.tile([P, 3, CIC, CO], BF16, name="w6")
                b6 = sb5.tile([CO, 3], F32, name="b6", padded_shape=[P, 3])
                for j in range(3):
                    for cic in range(CIC):
                        nc.sync.dma_start(w6[:, j, cic, :], w678g[j, cic])
                for j, bsrc in enumerate((B8, B6, B7)):
                    nc.sync.dma_start(b6[:, j: j + 1], bsrc[:])
                for oi, src in enumerate((fsum, sa_conv, sc_conv)):
                    for hf in range(2):
                        hp = ph5.tile([CO, 450], F32, name="hp", tag="hp",
                                      padded_shape=[P, 450])
                        for cic in range(CIC):
                            nc.tensor.matmul(
                                hp, w6[:, oi, cic, :],
                                src[cic][:, hf * 450: (hf + 1) * 450],
                                start=cic == 0, stop=cic == CIC - 1,
                            )
                        osb = sb5.tile([CO, 450], F32, name="osb", tag="osb",
                                       padded_shape=[P, 450])
                        nc.vector.tensor_scalar_add(osb[:], hp[:], b6[:, oi: oi + 1])
                        nc.sync.dma_start(OUT[oi, :, hf * 450: (hf + 1) * 450], osb[:])
            ctx.close()

    if split:
        _split_waits(nc)
    return nc


# ------------------------------------------------------------- host side ---

def _bn_fold(p):
    s, b, m, v = np.asarray(p, np.float32)
    a = s / np.sqrt(v + EPS)
    return a.astype(np.float32), (b - m * a).astype(np.float32)


def _bn_layout(a, b):
    # [P, 2, CIC]: [:, 0, c] = a-slice c, [:, 1, c] = b-slice c
    st = np.stack([a.reshape(CIC, P), b.reshape(CIC, P)])   # [2, CIC, P]
    return np.ascontiguousarray(st.transpose(2, 0, 1).astype(np.float32))


def host_prep(inputs):
    """Build the 8 per-core input maps."""
    inp = {k: np.asarray(v) for k, v in inputs.items()}
    x = inp["x"].astype(np.float32)
    d = inp["d"].astype(np.float32)
    lam = np.float32(inp["lamb"])
    B = x.shape[0]

    def w0_blocks(w):
        # [O=512, I=2048, 3, 3] -> per in-slice s: [CIC, P, 9, CI] lhsT
        out = []
        for s in range(4):
            ws = w[:, s * CI:(s + 1) * CI]            # [512, 512, 3, 3]
            t = np.transpose(ws, (1, 2, 3, 0))        # [I, 3, 3, O]
            out.append(np.ascontiguousarray(
                t.reshape(CIC, P, 9, CI).astype(bf)))
        return out

    def conv_w_full(w):
        # [512, 512, 3, 3] -> [9, 4, 128, 512]
        t = np.transpose(w, (2, 3, 1, 0))             # [3,3,512,512]
        return np.ascontiguousarray(t.reshape(9, CIC, P, CI).astype(bf))

    blk_s = w0_blocks(inp["w_s0"])
    blk_c = w0_blocks(inp["w_c0"])

    a0s, b0s = _bn_fold(inp["bn_s0"])
    a0c, b0c = _bn_fold(inp["bn_c0"])
    a1s, b1s = _bn_fold(inp["bn_s1"])
    a1c, b1c = _bn_fold(inp["bn_c1"])
    bn0s = _bn_layout(a0s, b0s)
    bn0c = _bn_layout(a0c, b0c)
    bn1s = _bn_layout(a1s, b1s)
    bn1c = _bn_layout(a1c, b1c)

    wqt = inp["wq"].T.reshape(CIC, P, CQ).astype(np.float32)
    wkt = inp["wk"].T.reshape(CIC, P, CQ).astype(np.float32)
    wqk8 = np.ascontiguousarray(np.stack([wqt, wkt])).reshape(8, QCH)
    wvt8 = np.ascontiguousarray(
        inp["wv"].T.reshape(CIC, P, CI).astype(bf)).reshape(8, VCH)
    w1s8 = conv_w_full(inp["w_s1"]).reshape(8, WCH)
    w1c8 = conv_w_full(inp["w_c1"]).reshape(8, WCH)
    w6t = inp["w6"].T.reshape(CIC, P, CO).astype(bf)
    w7t = inp["w7"].T.reshape(CIC, P, CO).astype(bf)
    w8t = inp["w8"].T.reshape(CIC, P, CO).astype(bf)
    w678_8 = np.ascontiguousarray(np.stack([w8t, w6t, w7t])).reshape(8, OCH)

    gsa = np.full((1, P), np.float32(inp["gamma_sa"]), np.float32)
    gsc = np.full((P, 1), np.float32(inp["gamma_sc"]), np.float32)

    in_maps = []
    for c in range(8):
        b_, s = c // 4, c % 4
        df = d[b_, 0].reshape(NPIX)
        dka = np.zeros((2, MP), np.float32)
        dka[0, :NPIX] = lam * df * df
        dka[0, NPIX:] = -1000.0
        dka[1, :NPIX] = df

        out_r0 = 15 * s
        dqa = np.zeros((2, WINP), np.float32)
        qmask = np.zeros((1, WINP), np.float32)
        dqa[0, :WIN] = 1.0
        for v_ in range(17):
            rv = out_r0 - 1 + v_
            if 0 <= rv < H:
                dqa[1, v_ * H:(v_ + 1) * H] = -2.0 * lam * d[b_, 0, rv]
                qmask[0, v_ * H:(v_ + 1) * H] = 1.0

        half = slice(0, 2) if c < 4 else slice(2, 4)
        in_maps.append({
            "XP": np.ascontiguousarray(
                x[b_, s * CI:(s + 1) * CI].reshape(CIC, P, NPIX).astype(bf)),
            "W0SH": np.ascontiguousarray(blk_s[s][half]),
            "W0CH": np.ascontiguousarray(blk_c[s][half]),
            "BN0S": bn0s, "BN0C": bn0c,
            "W1SP": w1s8[c], "W1CP": w1c8[c],
            "WVTP": wvt8[c], "W678P": w678_8[c], "WQKP": wqk8[c],
            "BQ": inp["bq"].reshape(CQ, 1).astype(np.float32),
            "BK": inp["bk"].reshape(CQ, 1).astype(np.float32),
            "BV": np.ascontiguousarray(
                inp["bv"].reshape(CIC, P).T.astype(np.float32)),
            "DKA": dka, "DQA": dqa, "QMASK": qmask,
            "GSA": gsa, "GSC": gsc,
            "BN1S": bn1s, "BN1C": bn1c,
            "B6": inp["b6"].reshape(CO, 1).astype(np.float32),
            "B7": inp["b7"].reshape(CO, 1).astype(np.float32),
            "B8": inp["b8"].reshape(CO, 1).astype(np.float32),
        })
    return in_maps


def assemble(results):
    """results: list of 8 dicts with 'OUT' [3, 40, 900] -> output tuple."""
    outs = []
    for b_ in range(2):
        rows = [np.asarray(results[4 * b_ + s]["OUT"], np.float32).reshape(
            3, CO, 15, H) for s in range(4)]
        outs.append(np.concatenate(rows, axis=2))        # [3, 40, 60, 60]
    full = np.stack(outs, axis=1)                        # [3, B, 40, 60, 60]
    return full[0], full[1], full[2]


def _split_waits(nc, keep=1):
    """Walrus in this container accepts at most one embedded sync-wait per
    instruction; Tile emits several. Turn extra waits into standalone
    single-wait EventSemaphore instructions before the owner, same engine."""
    n_split = 0
    for fn in nc.m.functions:
        for bb in fn.blocks:
            new_insts = []
            for inst in bb.instructions:
                si = inst.sync_info
                if si is not None and len(si.on_wait) > keep:
                    waits = list(si.on_wait)
                    head, tail = waits[:-keep], waits[-keep:]
                    for j, w in enumerate(head):
                        new_insts.append(mybir.InstEventSemaphore(
                            name=f"{inst.name}-ws{j}",
                            engine=inst.engine,
                            ins=[], outs=[],
                            sync_info=mybir.SyncInfo(on_wait=[w], on_update=[]),
                        ))
                        n_split += 1
                    inst.sync_info = mybir.SyncInfo(
                        on_wait=tail, on_update=list(si.on_update))
                new_insts.append(inst)
            bb.instructions.clear()
            bb.instructions.extend(new_insts)
    return n_split


_NC = None


def kernel(**inputs):
    global _NC
    if _NC is None:
        _NC = build_nc()
    from concourse.bass_utils import run_bass_kernel_spmd
    in_maps = host_prep(inputs)
    res = run_bass_kernel_spmd(_NC, in_maps, core_ids=list(range(8)))
    return assemble(res.results)


# revision 24
# speedup vs baseline: 1.1461x; 1.1461x over previous
"""DANetHead (dual attention head) Trainium2 kernel.

Strategy (8 NeuronCores): 2-way data parallel over batch B=2 (core groups
[0-3], [4-7]) x 4-way model parallel within each batch group.

Host->device traffic is minimized (it dominates the graded wall time):
  - x is channel-split: each core receives 512 of 2048 input channels
    (unreplicated). Stage-1 3x3 convs compute partial sums over those
    channels for ALL 512 output channels; an AllReduce over the batch
    group forms the true pre-activation, and bn_relu is applied locally.
  - Stage-1 conv weights: cores c and c+4 need identical blocks, so each
    receives half and an AllGather over pairs [[0,4],...] reconstructs.
  - Stage-2 conv weights + attention weights (identical on all 8 cores)
    are shipped 1/8th per core and AllGathered over all 8.

Matmuls run in bf16 (f32 PSUM accumulation) except the attention/CAM logits
which use f32 / hi-lo bf16 splitting to keep softmax inputs accurate.
"""

import os
from contextlib import ExitStack

import numpy as np
import ml_dtypes

import concourse.bass as bass
import concourse.tile as tile
import concourse.mybir as mybir
from concourse.bass import ds

dt = mybir.dt
F32 = dt.float32
BF16 = dt.bfloat16
AF = mybir.ActivationFunctionType
AX = mybir.AxisListType
ALU = mybir.AluOpType

P = 128
H = 60
HP = 62
NPIX = 3600          # 60*60
NPAD = 3720          # 60 zero + 3600 + 60 zero (padded full feature map)
MP = 3712            # 29*128, padded key/value pixel count
MCH = 29             # m chunks
WIN = 1020           # 17 rows * 60 query window
WINP = 1024          # padded window
CIN = 2048
CI = 512
CIC = 4              # 512 / 128
CQ = 64
CO = 40
CSH = 25.0           # softmax shift constant (max logit ~24.8)
GROUPS = [[0, 1, 2, 3], [4, 5, 6, 7]]
PAIRS = [[0, 4], [1, 5], [2, 6], [3, 7]]
WCH = 9 * CIC * P * CI // 8      # 294912  (1/8 of a stage-2 conv weight)
VCH = CIC * P * CI // 8          # 32768
OCH = 3 * CIC * P * CO // 8      # 7680
QCH = 2 * CIC * P * CQ // 8      # 8192
EPS = 1e-5

bf = ml_dtypes.bfloat16
_SKIP_CC = bool(int(os.environ.get("DANET_SKIP_CC", "0")))


# ---------------------------------------------------------------- builder ---

def build_nc(split=True, reps=1):
    nc = bass.Bass(num_devices=8)

    # ---- inputs (per-core contents differ; shapes identical) ----
    # XP: this core's 256 input channels for BOTH batches [batch, cic2, P, pix]
    XP = nc.dram_tensor("XP", [2, 2, P, NPIX], BF16, kind="ExternalInput")
    # W0*H: weights for those 256 channels -> all 512 outs (batch-independent,
    # so they serve both batches locally; no weight exchange needed)
    W0SH = nc.dram_tensor("W0SH", [2, P, 9, CI], BF16, kind="ExternalInput")
    W0CH = nc.dram_tensor("W0CH", [2, P, 9, CI], BF16, kind="ExternalInput")
    BN0S = nc.dram_tensor("BN0S", [P, 2, CIC], F32, kind="ExternalInput")
    BN0C = nc.dram_tensor("BN0C", [P, 2, CIC], F32, kind="ExternalInput")
    W1SP = nc.dram_tensor("W1SP", [WCH], BF16, kind="ExternalInput")
    W1CP = nc.dram_tensor("W1CP", [WCH], BF16, kind="ExternalInput")
    WVTP = nc.dram_tensor("WVTP", [VCH], BF16, kind="ExternalInput")
    W678P = nc.dram_tensor("W678P", [OCH], BF16, kind="ExternalInput")
    WQKP = nc.dram_tensor("WQKP", [QCH], F32, kind="ExternalInput")
    BQ = nc.dram_tensor("BQ", [CQ, 1], F32, kind="ExternalInput")
    BK = nc.dram_tensor("BK", [CQ, 1], F32, kind="ExternalInput")
    BV = nc.dram_tensor("BV", [P, CIC], F32, kind="ExternalInput")
    DKA = nc.dram_tensor("DKA", [2, MP], F32, kind="ExternalInput")
    DQA = nc.dram_tensor("DQA", [2, WINP], F32, kind="ExternalInput")
    QMASK = nc.dram_tensor("QMASK", [1, WINP], F32, kind="ExternalInput")
    GSA = nc.dram_tensor("GSA", [1, P], F32, kind="ExternalInput")
    GSC = nc.dram_tensor("GSC", [P, 1], F32, kind="ExternalInput")
    BN1S = nc.dram_tensor("BN1S", [P, 2, CIC], F32, kind="ExternalInput")
    BN1C = nc.dram_tensor("BN1C", [P, 2, CIC], F32, kind="ExternalInput")
    B6 = nc.dram_tensor("B6", [CO, 1], F32, kind="ExternalInput")
    B7 = nc.dram_tensor("B7", [CO, 1], F32, kind="ExternalInput")
    B8 = nc.dram_tensor("B8", [CO, 1], F32, kind="ExternalInput")
    OUT = nc.dram_tensor("OUT", [3, CO, 900], F32, kind="ExternalOutput")

    with tile.TileContext(nc) as tc:
        for _rep in range(reps):
            ctx = ExitStack()
            dram = ctx.enter_context(tc.tile_pool(name="dram", bufs=1, space="DRAM"))

            # window start within the padded full features: 900 * (core % 4)
            woff = (nc.sync.partition_id() % 4) * 900
            # my batch's offset within the two-batch reduced partials
            boff = (nc.sync.partition_id() // 4) * NPIX

            # gathered weights
            wqkg = dram.tile([2, CIC, P, CQ], F32, name="wqkg")
            wvg = dram.tile([CIC, P, CI], BF16, name="wvg")
            w1sg = dram.tile([9, CIC, P, CI], BF16, name="w1sg")
            w1cg = dram.tile([9, CIC, P, CI], BF16, name="w1cg")
            w678g = dram.tile([3, CIC, P, CO], BF16, name="w678g")

            # collectives cannot read IO tensors: stage inputs in DRAM tiles
            ALL8 = [[0, 1, 2, 3, 4, 5, 6, 7]]

            def ag(groups, src, shape, dtype, out_tile, name):
                st = dram.tile(shape, dtype, name=name)
                nc.sync.dma_start(st[:], src[:])
                nc.gpsimd.collective_compute(
                    "AllGather", ALU.bypass, replica_groups=groups,
                    ins=[st.opt()], outs=[out_tile.opt()],
                )

            ag(ALL8, WQKP, [QCH], F32, wqkg, "wqkp_st")
            ag(ALL8, WVTP, [VCH], BF16, wvg, "wvtp_st")
            ag(ALL8, W1SP, [WCH], BF16, w1sg, "w1sp_st")
            ag(ALL8, W1CP, [WCH], BF16, w1cg, "w1cp_st")
            ag(ALL8, W678P, [OCH], BF16, w678g, "w678p_st")

            # partial conv pre-activations (both batches) and reduced versions
            p1 = dram.tile([CIC, P, 2 * NPIX], BF16, name="p1")
            p2 = dram.tile([CIC, P, 2 * NPIX], BF16, name="p2")
            p1r = dram.tile([CIC, P, 2 * NPIX], BF16, name="p1r")
            p2r = dram.tile([CIC, P, 2 * NPIX], BF16, name="p2r")
            f1g = dram.tile([CIC, P, NPAD], F32, name="f1g")
            f2g = dram.tile([CIC, P, NPAD], F32, name="f2g")
            cen_in = dram.tile([CIC, P, CI], F32, name="cen_in")
            cen_out = dram.tile([CIC, P, CI], F32, name="cen_out")

            # =========== stage 1: partial 3x3 convs (512 ins -> 512 outs) =======
            with ExitStack() as c1:
                sb1 = c1.enter_context(tc.tile_pool(name="sb1", bufs=1))
                fb1 = c1.enter_context(tc.tile_pool(name="fb1", bufs=2))
                pp1 = c1.enter_context(tc.tile_pool(name="pp1", bufs=8, space="PSUM"))

                zsb = sb1.tile([P, H], F32, name="zsb")
                nc.any.memset(zsb[:], 0.0)
                for fg_ in (f1g, f2g):
                    for cic in range(CIC):
                        nc.sync.dma_start(fg_[cic, :, 0:H], zsb[:])
                        nc.sync.dma_start(fg_[cic, :, NPAD - H: NPAD], zsb[:])

                xpad = sb1.tile([P, 2, 2, HP * HP], BF16, name="xpad")
                nc.any.memset(xpad[:], 0.0)
                for b in range(2):
                    for c2 in range(2):
                        nc.sync.dma_start(
                            xpad[:, b, c2, :].rearrange(
                                "p (r c) -> p r c", c=HP)[:, 1:61, 1:61],
                            XP[b, c2].rearrange("p (r c) -> p r c", c=H),
                        )

                w0s_sb = [sb1.tile([P, 9, CI], BF16, name=f"w0s{i}")
                          for i in range(2)]
                w0c_sb = [sb1.tile([P, 9, CI], BF16, name=f"w0c{i}")
                          for i in range(2)]
                for c2 in range(2):
                    nc.sync.dma_start(w0s_sb[c2][:], W0SH[c2])
                    nc.sync.dma_start(w0c_sb[c2][:], W0CH[c2])

                for wsb, pdst, prd in ((w0s_sb, p1, p1r), (w0c_sb, p2, p2r)):
                    for b in range(2):
                        for ot in range(CIC):
                            pts = [
                                pp1.tile([P, 480], F32, name="s1p", tag="s1p")
                                for _ in range(8)
                            ]
                            for c2 in range(2):
                                xv = xpad[:, b, c2, :].rearrange(
                                    "p (r c) -> p r c", c=HP)
                                for off in range(9):
                                    ky, kx = off // 3, off % 3
                                    start = c2 == 0 and off == 0
                                    stop = c2 == 1 and off == 8
                                    for t in range(8):
                                        rows = 8 if t < 7 else 4
                                        rhs = xv[:, ky + 8 * t: ky + 8 * t + rows,
                                                 kx: kx + H]
                                        nc.tensor.matmul(
                                            pts[t][:, : rows * H],
                                            wsb[c2][:, off, ot * P: (ot + 1) * P],
                                            rhs, start=start, stop=stop,
                                        )
                            fbuf = fb1.tile([P, NPIX], BF16, name="fbuf", tag="fbuf")
                            for t in range(8):
                                rows = 8 if t < 7 else 4
                                nc.scalar.activation(
                                    fbuf[:, t * 480: t * 480 + rows * H],
                                    pts[t][:, : rows * H], AF.Copy,
                                )
                            nc.sync.dma_start(
                                pdst[ot, :, b * NPIX: (b + 1) * NPIX], fbuf[:])
                    nc.gpsimd.collective_compute(
                        "AllReduce", ALU.add, replica_groups=ALL8,
                        ins=[pdst.opt()], outs=[prd.opt()],
                    )

            # ====================== phase 2: bn_relu, windows, k, q, v ==========
            pers = ctx.enter_context(tc.tile_pool(name="pers", bufs=1))
            mid = ctx.enter_context(tc.tile_pool(name="mid", bufs=1))

            bn0s = pers.tile([P, 2, CIC], F32, name="bn0s")
            bn0c = pers.tile([P, 2, CIC], F32, name="bn0c")
            nc.sync.dma_start(bn0s[:], BN0S[:])
            nc.sync.dma_start(bn0c[:], BN0C[:])

            wqt = [pers.tile([P, CQ], F32, name=f"wqt{i}") for i in range(CIC)]
            wkt = [pers.tile([P, CQ], F32, name=f"wkt{i}") for i in range(CIC)]
            wvt = [pers.tile([P, CI], BF16, name=f"wvt{i}") for i in range(CIC)]
            for i in range(CIC):
                nc.sync.dma_start(wqt[i][:], wqkg[0, i])
                nc.sync.dma_start(wkt[i][:], wqkg[1, i])
                nc.sync.dma_start(wvt[i][:], wvg[i])
            bq = pers.tile([CQ, 1], F32, name="bq", padded_shape=[P, 1])
            bk = pers.tile([CQ, 1], F32, name="bk", padded_shape=[P, 1])
            bv = pers.tile([P, CIC], F32, name="bv")
            nc.sync.dma_start(bq[:], BQ[:])
            nc.sync.dma_start(bk[:], BK[:])
            nc.sync.dma_start(bv[:], BV[:])
            gsa = pers.tile([1, P], F32, name="gsa", padded_shape=[P, P])
            gsc = pers.tile([P, 1], F32, name="gsc")
            qmask = pers.tile([1, WINP], F32, name="qmask", padded_shape=[P, WINP])
            nc.sync.dma_start(gsa[:], GSA[:])
            nc.sync.dma_start(gsc[:], GSC[:])
            nc.sync.dma_start(qmask[:], QMASK[:])

            ka = mid.tile([P, MP], F32, name="ka")
            qa = mid.tile([P, WINP], F32, name="qa")
            kah = mid.tile([P, MP], BF16, name="kah")
            kal = mid.tile([P, MP], BF16, name="kal")
            qah = mid.tile([P, WINP], BF16, name="qah")
            qal = mid.tile([P, WINP], BF16, name="qal")
            nc.any.memset(ka[:], 0.0)
            nc.any.memset(qa[:], 0.0)
            nc.sync.dma_start(ka[64:66, :], DKA[:])
            nc.sync.dma_start(qa[64:66, :], DQA[:])

            f1w4 = pers.tile([P, CIC, WINP], F32, name="f1w4")
            f2w4 = pers.tile([P, CIC, WINP], F32, name="f2w4")
            f1win = [f1w4[:, i] for i in range(CIC)]
            f2win = [f2w4[:, i] for i in range(CIC)]
            vt = [pers.tile([P, MCH, P], BF16, name=f"vt{i}") for i in range(CIC)]
            pmy2 = dram.tile([CIC, P, NPIX], BF16, name="pmy2")

            with ExitStack() as c2:
                sb2 = c2.enter_context(tc.tile_pool(name="sb2", bufs=1))
                rp2 = c2.enter_context(tc.tile_pool(name="rp2", bufs=1))
                pk = c2.enter_context(tc.tile_pool(name="pk", bufs=8, space="PSUM"))

                vsp = c2.enter_context(tc.tile_pool(name="vsp", bufs=2))
                fh4 = sb2.tile([P, CIC, NPIX], BF16, name="fh4")
                f1h = [fh4[:, i] for i in range(CIC)]
                kps = [pk.tile([CQ, 450], F32, name="kp", tag="kp",
                               padded_shape=[P, 450]) for _ in range(8)]
                # one dynamic DMA pulls my batch's pre-act for all 4 chunks
                nc.sync.dma_start(
                    fh4[:],
                    p1r[:, :, ds(boff, NPIX)].rearrange("c p n -> p c n"))
                # f2's slice staged through DRAM (frees SP registers)
                nc.sync.dma_start(pmy2[:], p2r[:, :, ds(boff, NPIX)])
                for cic in range(CIC):
                    r32 = rp2.tile([P, NPIX], F32, name="r32", tag="r32")
                    nc.scalar.activation(
                        r32[:], f1h[cic][:], AF.Relu,
                        bias=bn0s[:, 1, cic: cic + 1],
                        scale=bn0s[:, 0, cic: cic + 1],
                    )
                    nc.sync.dma_start(f1g[cic, :, H: H + NPIX], r32[:])
                    nc.vector.tensor_copy(f1h[cic][:], r32[:])
                    for nt in range(8):
                        nc.tensor.matmul(
                            kps[nt], wkt[cic][:], r32[:, nt * 450: (nt + 1) * 450],
                            start=cic == 0, stop=cic == CIC - 1,
                        )
                for nt in range(8):
                    nc.vector.tensor_scalar_add(
                        ka[0:CQ, nt * 450: (nt + 1) * 450], kps[nt], bk[:]
                    )

                # f2 = bn_relu(reduced partials), written to padded map
                for cic in range(CIC):
                    rb2 = rp2.tile([P, NPIX], BF16, name="rb2", tag="rb2")
                    nc.sync.dma_start(rb2[:], pmy2[cic])
                    r32 = rp2.tile([P, NPIX], F32, name="r32", tag="r32")
                    nc.scalar.activation(
                        r32[:], rb2[:], AF.Relu,
                        bias=bn0c[:, 1, cic: cic + 1],
                        scale=bn0c[:, 0, cic: cic + 1],
                    )
                    nc.sync.dma_start(f2g[cic, :, H: H + NPIX], r32[:])

                # per-core windows (rows 15s-1 .. 15s+16 incl. halo);
                # single dynamic DMA per map (SP registers are scarce)
                nc.any.memset(f1w4[:], 0.0)
                nc.any.memset(f2w4[:], 0.0)
                nc.sync.dma_start(
                    f1w4[:, :, 0:WIN],
                    f1g[:, :, ds(woff, WIN)].rearrange("c p w -> p c w"))
                nc.sync.dma_start(
                    f2w4[:, :, 0:WIN],
                    f2g[:, :, ds(woff, WIN)].rearrange("c p w -> p c w"))

                # q from the f32 window
                for hf in range(2):
                    qp = pk.tile([CQ, 512], F32, name="qp", tag="kp",
                                 padded_shape=[P, 512])
                    for cic in range(CIC):
                        nc.tensor.matmul(
                            qp, wqt[cic][:], f1win[cic][:, hf * 512: (hf + 1) * 512],
                            start=cic == 0, stop=cic == CIC - 1,
                        )
                    nc.vector.tensor_scalar_add(
                        qa[0:CQ, hf * 512: (hf + 1) * 512], qp, bq[:]
                    )

                # v = wv @ f1 (bf16), then transpose
                for cot in range(CIC):
                    vsb = vsp.tile([P, MP], BF16, name="vsb", tag="vsb")
                    nc.any.memset(vsb[:, NPIX:MP], 0.0)
                    for nt in range(8):
                        vp = pk.tile([P, 450], F32, name="vp", tag="kp")
                        for cic in range(CIC):
                            nc.tensor.matmul(
                                vp,
                                wvt[cic][:, cot * P: (cot + 1) * P],
                                f1h[cic][:, nt * 450: (nt + 1) * 450],
                                start=cic == 0, stop=cic == CIC - 1,
                            )
                        nc.vector.tensor_scalar_add(
                            vsb[:, nt * 450: (nt + 1) * 450], vp, bv[:, cot: cot + 1]
                        )
                    nc.sync.dma_start_transpose(vt[cot][:], vsb[:])

            # hi/lo packing for the energy matmul:
            #   mm1: lhsT=[kh(64); aug(2); 0] rhs=[qh(64); augq(2); 0]
            #   mm2: lhsT=[kl(64); kh(64)]    rhs=[qh(64); ql(64)]
            nc.vector.memset(kah[:], 0.0)
            nc.vector.memset(qah[:], 0.0)
            nc.vector.tensor_copy(kah[0:66, :], ka[0:66, :])
            nc.vector.tensor_sub(kal[0:64, :], ka[0:64, :], kah[0:64, :])
            nc.vector.tensor_copy(kal[64:128, :], kah[0:64, :])
            nc.vector.tensor_copy(qah[0:66, :], qa[0:66, :])
            nc.vector.tensor_sub(qal[64:128, :], qa[0:64, :], qah[0:64, :])
            nc.vector.tensor_copy(qal[0:64, :], qah[0:64, :])

            # ================= phase 4a: CAM gram matrix (overlaps AR) ===========
            xfwin = [pers.tile([P, WINP], BF16, name=f"xfwin{i}") for i in range(CIC)]
            cen_sb = [mid.tile([P, CI], F32, name=f"cen{i}") for i in range(CIC)]
            with ExitStack() as c4:
                sb4 = c4.enter_context(tc.tile_pool(name="sb4", bufs=1))
                pc = c4.enter_context(tc.tile_pool(name="pc", bufs=2, space="PSUM"))
                xfh = sb4.tile([P, CIC, WINP], BF16, name="xfh")
                xfl = sb4.tile([P, CIC, WINP], BF16, name="xfl")
                xth = sb4.tile([P, 8, CIC, P], BF16, name="xth")
                xtl = sb4.tile([P, 8, CIC, P], BF16, name="xtl")
                tmpf = sb4.tile([P, 900], F32, name="tmpf")
                for i in range(CIC):
                    nc.any.memset(xfwin[i][:], 0.0)
                    nc.vector.tensor_copy(xfwin[i][:, 0:WIN], f2win[i][:, 0:WIN])
                    nc.any.memset(xfh[:, i, 900:WINP], 0.0)
                    nc.any.memset(xfl[:, i, 900:WINP], 0.0)
                    # hi/lo split of my 900 pixels (window cols 60:960)
                    nc.vector.tensor_copy(xfh[:, i, 0:900], f2win[i][:, 60:960])
                    nc.vector.tensor_copy(tmpf[:], xfh[:, i, 0:900])
                    nc.vector.tensor_sub(xfl[:, i, 0:900], f2win[i][:, 60:960], tmpf[:])
                    nc.sync.dma_start_transpose(xth[:, :, i, :], xfh[:, i, :])
                    nc.sync.dma_start_transpose(xtl[:, :, i, :], xfl[:, i, :])
                for ct in range(CIC):
                    cp = pc.tile([P, CI], F32, name="cp", tag="cp")
                    n_mm = 0
                    for nch in range(8):
                        for lh, rh in ((xth, xth), (xth, xtl), (xtl, xth)):
                            nc.tensor.matmul(
                                cp, lh[:, nch, ct, :],
                                rh[:, nch, :, :].rearrange("p a b -> p (a b)"),
                                start=n_mm == 0, stop=n_mm == 23,
                            )
                            n_mm += 1
                    nc.scalar.activation(cen_sb[ct][:], cp[:], AF.Copy)
                    nc.sync.dma_start(cen_in[ct], cen_sb[ct][:])
                if not _SKIP_CC:
                    nc.gpsimd.collective_compute(
                        "AllReduce", ALU.add,
                        replica_groups=GROUPS,
                        ins=[cen_in.opt()], outs=[cen_out.opt()],
                    )
                else:
                    nc.sync.dma_start(cen_out[:], cen_in[:])

            # ======================= phase 3: position attention =================
            sa_win = [mid.tile([P, WINP], BF16, name=f"sawin{i}") for i in range(CIC)]
            with ExitStack() as c3:
                sb3 = c3.enter_context(tc.tile_pool(name="sb3", bufs=1))
                ap3 = c3.enter_context(tc.tile_pool(name="ap3", bufs=3))
                pe3 = c3.enter_context(tc.tile_pool(name="pe3", bufs=2, space="PSUM"))
                psa = c3.enter_context(tc.tile_pool(name="psa", bufs=4, space="PSUM"))
                psum3 = c3.enter_context(tc.tile_pool(name="psum3", bufs=2, space="PSUM"))

                ones = sb3.tile([P, 1], BF16, name="ones")
                nc.any.memset(ones[:], 1.0)
                nshift = sb3.tile([P, 1], F32, name="nshift")
                nc.any.memset(nshift[:], -CSH)
                for hf in range(2):
                    hsl = slice(hf * 512, (hf + 1) * 512)
                    saps = [
                        psa.tile([P, 512], F32, name="sap", tag="sap")
                        for _ in range(CIC)
                    ]
                    sums = psum3.tile([1, 512], F32, name="sums", tag="sums",
                                      padded_shape=[P, 512])
                    for mc in range(MCH):
                        ep = pe3.tile([P, 512], F32, name="ep", tag="ep")
                        nc.tensor.matmul(
                            ep, kah[:, mc * P: (mc + 1) * P], qah[:, hsl],
                            start=True, stop=False,
                        )
                        nc.tensor.matmul(
                            ep, kal[:, mc * P: (mc + 1) * P], qal[:, hsl],
                            start=False, stop=True,
                        )
                        at = ap3.tile([P, 512], BF16, name="at", tag="at")
                        nc.scalar.activation(at[:], ep[:], AF.Exp,
                                             bias=nshift[:], scale=1.0)
                        nc.tensor.matmul(
                            sums, ones[:], at[:], start=mc == 0, stop=mc == MCH - 1
                        )
                        for cot in range(CIC):
                            nc.tensor.matmul(
                                saps[cot], vt[cot][:, mc, :], at[:],
                                start=mc == 0, stop=mc == MCH - 1,
                            )
                    ssb = sb3.tile([1, 512], F32, name="ssb", tag="ssb",
                                   padded_shape=[P, 512])
                    nc.scalar.activation(ssb[:], sums[:], AF.Copy)
                    rec = sb3.tile([1, 512], F32, name="rec", tag="rec",
                                   padded_shape=[P, 512])
                    nc.vector.reciprocal(rec[:], ssb[:])
                    nc.vector.tensor_mul(rec[:], rec[:], qmask[:, hsl])
                    rbp = pe3.tile([P, 512], F32, name="rbp", tag="ep")
                    nc.tensor.matmul(rbp, gsa[:], rec[:], start=True, stop=True)
                    recb = sb3.tile([P, 512], F32, name="recb", tag="recb")
                    nc.scalar.activation(recb[:], rbp[:], AF.Copy)
                    for cot in range(CIC):
                        tmp3 = sb3.tile([P, 512], F32, name="tmp3", tag="tmp3")
                        nc.vector.tensor_mul(tmp3[:], saps[cot][:], recb[:])
                        nc.vector.tensor_add(
                            sa_win[cot][:, hsl], tmp3[:], f1win[cot][:, hsl]
                        )

            # =================== phase 4b: CAM softmax + attention ===============
            sc_win = [mid.tile([P, WINP], BF16, name=f"scwin{i}") for i in range(CIC)]
            with ExitStack() as c4b:
                sb4b = c4b.enter_context(tc.tile_pool(name="sb4b", bufs=1))
                pc2 = c4b.enter_context(tc.tile_pool(name="pc2", bufs=2, space="PSUM"))
                cattT = sb4b.tile([P, CIC, CIC, P], BF16, name="cattT")
                crec = sb4b.tile([P, CIC], F32, name="crec")
                for ct in range(CIC):
                    cg = cen_sb[ct]
                    nc.sync.dma_start(cg[:], cen_out[ct])
                    rmin = sb4b.tile([P, 1], F32, name="rmin", tag="rmin")
                    nc.vector.tensor_reduce(rmin[:], cg[:], axis=AX.X, op=ALU.min)
                    cat = sb4b.tile([P, CI], BF16, name="cat", tag="cat", bufs=2)
                    csum = sb4b.tile([P, 1], F32, name="csum", tag="csum", bufs=2)
                    nc.scalar.activation(
                        cat[:], cg[:], AF.Exp, bias=rmin[:], scale=-1.0,
                        accum_out=csum[:],
                    )
                    nc.vector.reciprocal(crec[:, ct: ct + 1], csum[:])
                    nc.vector.tensor_mul(crec[:, ct: ct + 1], crec[:, ct: ct + 1],
                                         gsc[:])
                    nc.sync.dma_start_transpose(cattT[:, :, ct, :], cat[:])
                for ct in range(CIC):
                    for hf in range(2):
                        hsl = slice(hf * 512, (hf + 1) * 512)
                        scp = pc2.tile([P, 512], F32, name="scp", tag="scp")
                        for dch in range(CIC):
                            nc.tensor.matmul(
                                scp, cattT[:, dch, ct, :], xfwin[dch][:, hsl],
                                start=dch == 0, stop=dch == CIC - 1,
                            )
                        tmp4 = sb4b.tile([P, 512], F32, name="tmp4", tag="tmp4")
                        nc.scalar.activation(tmp4[:], scp[:], AF.Copy,
                                             scale=crec[:, ct: ct + 1])
                        nc.vector.tensor_add(
                            sc_win[ct][:, hsl], tmp4[:], f2win[ct][:, hsl]
                        )

            # ============= phase 5: pads, stage-2 convs, output heads ============
            late = ctx.enter_context(tc.tile_pool(name="late", bufs=1))
            sa_pad = [late.tile([P, 17, HP], BF16, name=f"sapad{i}") for i in range(CIC)]
            sc_pad = [late.tile([P, 17, HP], BF16, name=f"scpad{i}") for i in range(CIC)]
            for i in range(CIC):
                nc.any.memset(sa_pad[i][:], 0.0)
                nc.any.memset(sc_pad[i][:], 0.0)
                nc.vector.tensor_copy(
                    sa_pad[i][:, :, 1:61],
                    sa_win[i][:, 0:WIN].rearrange("p (r c) -> p r c", c=H),
                )
                nc.vector.tensor_copy(
                    sc_pad[i][:, :, 1:61],
                    sc_win[i][:, 0:WIN].rearrange("p (r c) -> p r c", c=H),
                )

            sa_conv = [late.tile([P, 900], BF16, name=f"sacv{i}") for i in range(CIC)]
            sc_conv = [late.tile([P, 900], BF16, name=f"sccv{i}") for i in range(CIC)]
            fsum = [late.tile([P, 900], BF16, name=f"fsum{i}") for i in range(CIC)]

            with ExitStack() as c5:
                sb5 = c5.enter_context(tc.tile_pool(name="sb5", bufs=1))
                wp5 = c5.enter_context(tc.tile_pool(name="wp5", bufs=4))
                pp5 = c5.enter_context(tc.tile_pool(name="pp5", bufs=3, space="PSUM"))
                ph5 = c5.enter_context(tc.tile_pool(name="ph5", bufs=2, space="PSUM"))

                bn1 = sb5.tile([P, 2, 2, CIC], F32, name="bn1")
                nc.sync.dma_start(bn1[:, 0], BN1S[:])
                nc.sync.dma_start(bn1[:, 1], BN1C[:])

                for bi, (wsrc, pad, cv) in enumerate(
                    ((w1sg, sa_pad, sa_conv), (w1cg, sc_pad, sc_conv))
                ):
                    for cot in range(CIC):
                        cps = [
                            pp5.tile([P, 300], F32, name="cp5", tag="cp5")
                            for _ in range(3)
                        ]
                        for cic in range(CIC):
                            wt9 = wp5.tile([P, 9, P], BF16, name="w1t", tag="w1t")
                            nc.sync.dma_start(
                                wt9[:],
                                wsrc[:, cic, :, cot * P: (cot + 1) * P]
                                .rearrange("o p q -> p o q"))
                            for off in range(9):
                                ky, kx = off // 3, off % 3
                                start = cic == 0 and off == 0
                                stop = cic == CIC - 1 and off == 8
                                for rt in range(3):
                                    rhs = pad[cic][
                                        :, rt * 5 + ky: rt * 5 + ky + 5, kx: kx + H
                                    ]
                                    nc.tensor.matmul(
                                        cps[rt], wt9[:, off, :], rhs,
                                        start=start, stop=stop
                                    )
                        for rt in range(3):
                            nc.scalar.activation(
                                cv[cot][:, rt * 300: (rt + 1) * 300], cps[rt][:],
                                AF.Relu, bias=bn1[:, bi, 1, cot: cot + 1],
                                scale=bn1[:, bi, 0, cot: cot + 1],
                            )
                for i in range(CIC):
                    nc.vector.tensor_add(fsum[i][:], sa_conv[i][:], sc_conv[i][:])

                w6 = sb5.tile([P, 3, CIC, CO], BF16, name="w6")
                b6 = sb5.tile([CO, 3], F32, name="b6", padded_shape=[P, 3])
                for j in range(3):
                    for cic in range(CIC):
                        nc.sync.dma_start(w6[:, j, cic, :], w678g[j, cic])
                for j, bsrc in enumerate((B8, B6, B7)):
                    nc.sync.dma_start(b6[:, j: j + 1], bsrc[:])
                for oi, src in enumerate((fsum, sa_conv, sc_conv)):
                    for hf in range(2):
                        hp = ph5.tile([CO, 450], F32, name="hp", tag="hp",
                                      padded_shape=[P, 450])
                        for cic in range(CIC):
                            nc.tensor.matmul(
                                hp, w6[:, oi, cic, :],
                                src[cic][:, hf * 450: (hf + 1) * 450],
                                start=cic == 0, stop=cic == CIC - 1,
                            )
                        osb = sb5.tile([CO, 450], F32, name="osb", tag="osb",
                                       padded_shape=[P, 450])
                        nc.vector.tensor_scalar_add(osb[:], hp[:], b6[:, oi: oi + 1])
                        nc.sync.dma_start(OUT[oi, :, hf * 450: (hf + 1) * 450], osb[:])
            ctx.close()

    if split:
        _split_waits(nc)
    return nc


# ------------------------------------------------------------- host side ---

def _bn_fold(p):
    s, b, m, v = np.asarray(p, np.float32)
    a = s / np.sqrt(v + EPS)
    return a.astype(np.float32), (b - m * a).astype(np.float32)


def _bn_layout(a, b):
    # [P, 2, CIC]: [:, 0, c] = a-slice c, [:, 1, c] = b-slice c
    st = np.stack([a.reshape(CIC, P), b.reshape(CIC, P)])   # [2, CIC, P]
    return np.ascontiguousarray(st.transpose(2, 0, 1).astype(np.float32))


def host_prep(inputs):
    """Build the 8 per-core input maps."""
    inp = {k: np.asarray(v) for k, v in inputs.items()}
    x = inp["x"].astype(np.float32)
    d = inp["d"].astype(np.float32)
    lam = np.float32(inp["lamb"])
    B = x.shape[0]

    def w0_blocks(w):
        # [O=512, I=2048, 3, 3] -> per 256-ch slice c: [2, P, 9, CI] lhsT
        out = []
        for c in range(8):
            ws = w[:, c * 256:(c + 1) * 256]          # [512, 256, 3, 3]
            t = np.transpose(ws, (1, 2, 3, 0))        # [I256, 3, 3, O]
            out.append(np.ascontiguousarray(
                t.reshape(2, P, 9, CI).astype(bf)))
        return out

    def conv_w_full(w):
        # [512, 512, 3, 3] -> [9, 4, 128, 512]
        t = np.transpose(w, (2, 3, 1, 0))             # [3,3,512,512]
        return np.ascontiguousarray(t.reshape(9, CIC, P, CI).astype(bf))

    blk_s = w0_blocks(inp["w_s0"])
    blk_c = w0_blocks(inp["w_c0"])

    a0s, b0s = _bn_fold(inp["bn_s0"])
    a0c, b0c = _bn_fold(inp["bn_c0"])
    a1s, b1s = _bn_fold(inp["bn_s1"])
    a1c, b1c = _bn_fold(inp["bn_c1"])
    bn0s = _bn_layout(a0s, b0s)
    bn0c = _bn_layout(a0c, b0c)
    bn1s = _bn_layout(a1s, b1s)
    bn1c = _bn_layout(a1c, b1c)

    wqt = inp["wq"].T.reshape(CIC, P, CQ).astype(np.float32)
    wkt = inp["wk"].T.reshape(CIC, P, CQ).astype(np.float32)
    wqk8 = np.ascontiguousarray(np.stack([wqt, wkt])).reshape(8, QCH)
    wvt8 = np.ascontiguousarray(
        inp["wv"].T.reshape(CIC, P, CI).astype(bf)).reshape(8, VCH)
    w1s8 = conv_w_full(inp["w_s1"]).reshape(8, WCH)
    w1c8 = conv_w_full(inp["w_c1"]).reshape(8, WCH)
    w6t = inp["w6"].T.reshape(CIC, P, CO).astype(bf)
    w7t = inp["w7"].T.reshape(CIC, P, CO).astype(bf)
    w8t = inp["w8"].T.reshape(CIC, P, CO).astype(bf)
    w678_8 = np.ascontiguousarray(np.stack([w8t, w6t, w7t])).reshape(8, OCH)

    gsa = np.full((1, P), np.float32(inp["gamma_sa"]), np.float32)
    gsc = np.full((P, 1), np.float32(inp["gamma_sc"]), np.float32)

    in_maps = []
    for c in range(8):
        b_, s = c // 4, c % 4
        df = d[b_, 0].reshape(NPIX)
        dka = np.zeros((2, MP), np.float32)
        dka[0, :NPIX] = lam * df * df
        dka[0, NPIX:] = -1000.0
        dka[1, :NPIX] = df

        out_r0 = 15 * s
        dqa = np.zeros((2, WINP), np.float32)
        qmask = np.zeros((1, WINP), np.float32)
        dqa[0, :WIN] = 1.0
        for v_ in range(17):
            rv = out_r0 - 1 + v_
            if 0 <= rv < H:
                dqa[1, v_ * H:(v_ + 1) * H] = -2.0 * lam * d[b_, 0, rv]
                qmask[0, v_ * H:(v_ + 1) * H] = 1.0

        in_maps.append({
            "XP": np.ascontiguousarray(
                x[:, c * 256:(c + 1) * 256].reshape(2, 2, P, NPIX).astype(bf)),
            "W0SH": blk_s[c],
            "W0CH": blk_c[c],
            "BN0S": bn0s, "BN0C": bn0c,
            "W1SP": w1s8[c], "W1CP": w1c8[c],
            "WVTP": wvt8[c], "W678P": w678_8[c], "WQKP": wqk8[c],
            "BQ": inp["bq"].reshape(CQ, 1).astype(np.float32),
            "BK": inp["bk"].reshape(CQ, 1).astype(np.float32),
            "BV": np.ascontiguousarray(
                inp["bv"].reshape(CIC, P).T.astype(np.float32)),
            "DKA": dka, "DQA": dqa, "QMASK": qmask,
            "GSA": gsa, "GSC": gsc,
            "BN1S": bn1s, "BN1C": bn1c,
            "B6": inp["b6"].reshape(CO, 1).astype(np.float32),
            "B7": inp["b7"].reshape(CO, 1).astype(np.float32),
            "B8": inp["b8"].reshape(CO, 1).astype(np.float32),
        })
    return in_maps


def assemble(results):
    """results: list of 8 dicts with 'OUT' [3, 40, 900] -> output tuple."""
    outs = []
    for b_ in range(2):
        rows = [np.asarray(results[4 * b_ + s]["OUT"], np.float32).reshape(
            3, CO, 15, H) for s in range(4)]
        outs.append(np.concatenate(rows, axis=2))        # [3, 40, 60, 60]
    full = np.stack(outs, axis=1)                        # [3, B, 40, 60, 60]
    return full[0], full[1], full[2]


def _split_waits(nc, keep=1):
    """Walrus in this container accepts at most one embedded sync-wait per
    instruction; Tile emits several. Turn extra waits into standalone
    single-wait EventSemaphore instructions before the owner, same engine."""
    n_split = 0
    for fn in nc.m.functions:
        for bb in fn.blocks:
            new_insts = []
            for inst in bb.instructions:
                si = inst.sync_info
                if si is not None and len(si.on_wait) > keep:
                    waits = list(si.on_wait)
                    head, tail = waits[:-keep], waits[-keep:]
                    for j, w in enumerate(head):
                        new_insts.append(mybir.InstEventSemaphore(
                            name=f"{inst.name}-ws{j}",
                            engine=inst.engine,
                            ins=[], outs=[],
                            sync_info=mybir.SyncInfo(on_wait=[w], on_update=[]),
                        ))
                        n_split += 1
                    inst.sync_info = mybir.SyncInfo(
                        on_wait=tail, on_update=list(si.on_update))
                new_insts.append(inst)
            bb.instructions.clear()
            bb.instructions.extend(new_insts)
    return n_split


_NC = None


def kernel(**inputs):
    global _NC
    if _NC is None:
        _NC = build_nc()
    from concourse.bass_utils import run_bass_kernel_spmd
    in_maps = host_prep(inputs)
    res = run_bass_kernel_spmd(_NC, in_maps, core_ids=list(range(8)))
    return assemble(res.results)


# revision 36
# speedup vs baseline: 1.1494x; 1.0029x over previous
"""DANetHead (dual attention head) Trainium2 kernel.

Strategy (8 NeuronCores): 2-way data parallel over batch B=2 (core groups
[0-3], [4-7]) x 4-way model parallel within each batch group.

Host->device traffic is minimized (it dominates the graded wall time):
  - x is channel-split: each core receives 512 of 2048 input channels
    (unreplicated). Stage-1 3x3 convs compute partial sums over those
    channels for ALL 512 output channels; an AllReduce over the batch
    group forms the true pre-activation, and bn_relu is applied locally.
  - Stage-1 conv weights: cores c and c+4 need identical blocks, so each
    receives half and an AllGather over pairs [[0,4],...] reconstructs.
  - Stage-2 conv weights + attention weights (identical on all 8 cores)
    are shipped 1/8th per core and AllGathered over all 8.

Matmuls run in bf16 (f32 PSUM accumulation) except the attention/CAM logits
which use f32 / hi-lo bf16 splitting to keep softmax inputs accurate.
"""

import os
from contextlib import ExitStack

import numpy as np
import ml_dtypes

import concourse.bass as bass
import concourse.tile as tile
import concourse.mybir as mybir
from concourse.bass import ds

dt = mybir.dt
F32 = dt.float32
BF16 = dt.bfloat16
AF = mybir.ActivationFunctionType
AX = mybir.AxisListType
ALU = mybir.AluOpType

P = 128
H = 60
HP = 62
NPIX = 3600          # 60*60
NPAD = 3720          # 60 zero + 3600 + 60 zero (padded full feature map)
MP = 3712            # 29*128, padded key/value pixel count
MCH = 29             # m chunks
WIN = 1020           # 17 rows * 60 query window
WINP = 1024          # padded window
CIN = 2048
CI = 512
CIC = 4              # 512 / 128
CQ = 64
CO = 40
CSH = 25.0           # softmax shift constant (max logit ~24.8)
GROUPS = [[0, 1, 2, 3], [4, 5, 6, 7]]
PAIRS = [[0, 4], [1, 5], [2, 6], [3, 7]]
WCH = 9 * CIC * P * CI // 8      # 294912  (1/8 of a stage-2 conv weight)
VCH = CIC * P * CI // 8          # 32768
OCH = 3 * CIC * P * CO // 8      # 7680
QCH = 2 * CIC * P * CQ // 8      # 8192
EPS = 1e-5

bf = ml_dtypes.bfloat16
_SKIP_CC = bool(int(os.environ.get("DANET_SKIP_CC", "0")))


# ---------------------------------------------------------------- builder ---

def build_nc(split=True, reps=1):
    nc = bass.Bass(num_devices=8)

    # ---- inputs (per-core contents differ; shapes identical) ----
    # XP: this core's 256 input channels for BOTH batches [batch, cic2, P, pix]
    XP = nc.dram_tensor("XP", [2, 2, P, NPIX], BF16, kind="ExternalInput")
    # W0*H: weights for those 256 channels -> all 512 outs (batch-independent,
    # so they serve both batches locally; no weight exchange needed)
    W0SH = nc.dram_tensor("W0SH", [2, P, 9, CI], BF16, kind="ExternalInput")
    W0CH = nc.dram_tensor("W0CH", [2, P, 9, CI], BF16, kind="ExternalInput")
    BN0S = nc.dram_tensor("BN0S", [P, 2, CIC], F32, kind="ExternalInput")
    BN0C = nc.dram_tensor("BN0C", [P, 2, CIC], F32, kind="ExternalInput")
    W1SP = nc.dram_tensor("W1SP", [WCH], BF16, kind="ExternalInput")
    W1CP = nc.dram_tensor("W1CP", [WCH], BF16, kind="ExternalInput")
    WVTP = nc.dram_tensor("WVTP", [VCH], BF16, kind="ExternalInput")
    W678P = nc.dram_tensor("W678P", [OCH], BF16, kind="ExternalInput")
    WQKP = nc.dram_tensor("WQKP", [QCH], F32, kind="ExternalInput")
    BQ = nc.dram_tensor("BQ", [CQ, 1], F32, kind="ExternalInput")
    BK = nc.dram_tensor("BK", [CQ, 1], F32, kind="ExternalInput")
    BV = nc.dram_tensor("BV", [P, CIC], F32, kind="ExternalInput")
    DKA = nc.dram_tensor("DKA", [2, MP], F32, kind="ExternalInput")
    DQA = nc.dram_tensor("DQA", [2, WINP], F32, kind="ExternalInput")
    QMASK = nc.dram_tensor("QMASK", [1, WINP], F32, kind="ExternalInput")
    GSA = nc.dram_tensor("GSA", [1, P], F32, kind="ExternalInput")
    GSC = nc.dram_tensor("GSC", [P, 1], F32, kind="ExternalInput")
    BN1S = nc.dram_tensor("BN1S", [P, 2, CIC], F32, kind="ExternalInput")
    BN1C = nc.dram_tensor("BN1C", [P, 2, CIC], F32, kind="ExternalInput")
    B6 = nc.dram_tensor("B6", [CO, 1], F32, kind="ExternalInput")
    B7 = nc.dram_tensor("B7", [CO, 1], F32, kind="ExternalInput")
    B8 = nc.dram_tensor("B8", [CO, 1], F32, kind="ExternalInput")
    OUT = nc.dram_tensor("OUT", [3, CO, 900], BF16, kind="ExternalOutput")

    with tile.TileContext(nc) as tc:
        for _rep in range(reps):
            ctx = ExitStack()
            dram = ctx.enter_context(tc.tile_pool(name="dram", bufs=1, space="DRAM"))

            # window start within the padded full features: 900 * (core % 4)
            woff = (nc.sync.partition_id() % 4) * 900
            # my batch's offset within the two-batch reduced partials
            boff = (nc.sync.partition_id() // 4) * NPIX

            # gathered weights
            wqkg = dram.tile([2, CIC, P, CQ], F32, name="wqkg")
            wvg = dram.tile([CIC, P, CI], BF16, name="wvg")
            w1sg = dram.tile([9, CIC, P, CI], BF16, name="w1sg")
            w1cg = dram.tile([9, CIC, P, CI], BF16, name="w1cg")
            w678g = dram.tile([3, CIC, P, CO], BF16, name="w678g")

            # collectives cannot read IO tensors: stage inputs in DRAM tiles
            ALL8 = [[0, 1, 2, 3, 4, 5, 6, 7]]

            def ag(groups, src, shape, dtype, out_tile, name):
                st = dram.tile(shape, dtype, name=name)
                nc.sync.dma_start(st[:], src[:])
                nc.gpsimd.collective_compute(
                    "AllGather", ALU.bypass, replica_groups=groups,
                    ins=[st.opt()], outs=[out_tile.opt()],
                )

            ag(ALL8, WQKP, [QCH], F32, wqkg, "wqkp_st")
            ag(ALL8, WVTP, [VCH], BF16, wvg, "wvtp_st")
            ag(ALL8, W1SP, [WCH], BF16, w1sg, "w1sp_st")
            ag(ALL8, W1CP, [WCH], BF16, w1cg, "w1cp_st")
            ag(ALL8, W678P, [OCH], BF16, w678g, "w678p_st")

            # partial conv pre-activations (both batches) and reduced versions
            p1 = dram.tile([CIC, P, 2 * NPIX], BF16, name="p1")
            p2 = dram.tile([CIC, P, 2 * NPIX], BF16, name="p2")
            p1r = dram.tile([CIC, P, 2 * NPIX], BF16, name="p1r")
            p2r = dram.tile([CIC, P, 2 * NPIX], BF16, name="p2r")
            f1g = dram.tile([CIC, P, NPAD], F32, name="f1g")
            f2g = dram.tile([CIC, P, NPAD], F32, name="f2g")
            cen_in = dram.tile([CIC, P, CI], F32, name="cen_in")
            cen_out = dram.tile([CIC, P, CI], F32, name="cen_out")

            # =========== stage 1: partial 3x3 convs (512 ins -> 512 outs) =======
            with ExitStack() as c1:
                sb1 = c1.enter_context(tc.tile_pool(name="sb1", bufs=1))
                fb1 = c1.enter_context(tc.tile_pool(name="fb1", bufs=2))
                pp1 = c1.enter_context(tc.tile_pool(name="pp1", bufs=8, space="PSUM"))

                zsb = sb1.tile([P, H], F32, name="zsb")
                nc.any.memset(zsb[:], 0.0)
                for fg_ in (f1g, f2g):
                    for cic in range(CIC):
                        nc.sync.dma_start(fg_[cic, :, 0:H], zsb[:])
                        nc.sync.dma_start(fg_[cic, :, NPAD - H: NPAD], zsb[:])

                xpad = [[sb1.tile([P, HP * HP], BF16, name=f"xpad{b}{c2}")
                         for c2 in range(2)] for b in range(2)]
                for b in range(2):
                    for c2 in range(2):
                        nc.any.memset(xpad[b][c2][:], 0.0)
                        nc.sync.dma_start(
                            xpad[b][c2][:].rearrange(
                                "p (r c) -> p r c", c=HP)[:, 1:61, 1:61],
                            XP[b, c2].rearrange("p (r c) -> p r c", c=H),
                        )

                w0s_sb = [sb1.tile([P, 9, CI], BF16, name=f"w0s{i}")
                          for i in range(2)]
                w0c_sb = [sb1.tile([P, 9, CI], BF16, name=f"w0c{i}")
                          for i in range(2)]
                for c2 in range(2):
                    nc.sync.dma_start(w0s_sb[c2][:], W0SH[c2])
                    nc.sync.dma_start(w0c_sb[c2][:], W0CH[c2])

                for wsb, pdst, prd in ((w0s_sb, p1, p1r), (w0c_sb, p2, p2r)):
                    for b in range(2):
                        for ot in range(CIC):
                            pts = [
                                pp1.tile([P, 480], F32, name="s1p", tag="s1p")
                                for _ in range(8)
                            ]
                            for c2 in range(2):
                                xv = xpad[b][c2][:].rearrange(
                                    "p (r c) -> p r c", c=HP)
                                for off in range(9):
                                    ky, kx = off // 3, off % 3
                                    start = c2 == 0 and off == 0
                                    stop = c2 == 1 and off == 8
                                    for t in range(8):
                                        rows = 8 if t < 7 else 4
                                        rhs = xv[:, ky + 8 * t: ky + 8 * t + rows,
                                                 kx: kx + H]
                                        nc.tensor.matmul(
                                            pts[t][:, : rows * H],
                                            wsb[c2][:, off, ot * P: (ot + 1) * P],
                                            rhs, start=start, stop=stop,
                                        )
                            fbuf = fb1.tile([P, NPIX], BF16, name="fbuf", tag="fbuf")
                            for t in range(8):
                                rows = 8 if t < 7 else 4
                                nc.scalar.activation(
                                    fbuf[:, t * 480: t * 480 + rows * H],
                                    pts[t][:, : rows * H], AF.Copy,
                                )
                            nc.sync.dma_start(
                                pdst[ot, :, b * NPIX: (b + 1) * NPIX], fbuf[:])
                    nc.gpsimd.collective_compute(
                        "AllReduce", ALU.add, replica_groups=ALL8,
                        ins=[pdst.opt()], outs=[prd.opt()],
                    )

            # ====================== phase 2: bn_relu, windows, k, q, v ==========
            pers = ctx.enter_context(tc.tile_pool(name="pers", bufs=1))
            mid = ctx.enter_context(tc.tile_pool(name="mid", bufs=1))

            bn0s = pers.tile([P, 2, CIC], F32, name="bn0s")
            bn0c = pers.tile([P, 2, CIC], F32, name="bn0c")
            nc.sync.dma_start(bn0s[:], BN0S[:])
            nc.sync.dma_start(bn0c[:], BN0C[:])

            wqt = [pers.tile([P, CQ], F32, name=f"wqt{i}") for i in range(CIC)]
            wkt = [pers.tile([P, CQ], F32, name=f"wkt{i}") for i in range(CIC)]
            wvt = [pers.tile([P, CI], BF16, name=f"wvt{i}") for i in range(CIC)]
            for i in range(CIC):
                nc.sync.dma_start(wqt[i][:], wqkg[0, i])
                nc.sync.dma_start(wkt[i][:], wqkg[1, i])
                nc.sync.dma_start(wvt[i][:], wvg[i])
            bq = pers.tile([CQ, 1], F32, name="bq", padded_shape=[P, 1])
            bk = pers.tile([CQ, 1], F32, name="bk", padded_shape=[P, 1])
            bv = pers.tile([P, CIC], F32, name="bv")
            nc.sync.dma_start(bq[:], BQ[:])
            nc.sync.dma_start(bk[:], BK[:])
            nc.sync.dma_start(bv[:], BV[:])
            gsa = pers.tile([1, P], F32, name="gsa", padded_shape=[P, P])
            gsc = pers.tile([P, 1], F32, name="gsc")
            qmask = pers.tile([1, WINP], F32, name="qmask", padded_shape=[P, WINP])
            nc.sync.dma_start(gsa[:], GSA[:])
            nc.sync.dma_start(gsc[:], GSC[:])
            nc.sync.dma_start(qmask[:], QMASK[:])

            ka = mid.tile([P, MP], F32, name="ka")
            qa = mid.tile([P, WINP], F32, name="qa")
            nc.any.memset(ka[:], 0.0)
            nc.any.memset(qa[:], 0.0)
            nc.sync.dma_start(ka[64:66, :], DKA[:])
            nc.sync.dma_start(qa[64:66, :], DQA[:])

            f1w4 = pers.tile([P, CIC, WINP], F32, name="f1w4")
            f2w4 = pers.tile([P, CIC, WINP], F32, name="f2w4")
            f1win = [f1w4[:, i] for i in range(CIC)]
            f2win = [f2w4[:, i] for i in range(CIC)]
            vt = [pers.tile([P, MCH, P], BF16, name=f"vt{i}") for i in range(CIC)]
            pmy2 = dram.tile([CIC, P, NPIX], BF16, name="pmy2")

            with ExitStack() as c2:
                sb2 = c2.enter_context(tc.tile_pool(name="sb2", bufs=1))
                rp2 = c2.enter_context(tc.tile_pool(name="rp2", bufs=1))
                pk = c2.enter_context(tc.tile_pool(name="pk", bufs=8, space="PSUM"))

                vsp = c2.enter_context(tc.tile_pool(name="vsp", bufs=2))
                fh4 = sb2.tile([P, CIC, NPIX], BF16, name="fh4")
                f1h = [fh4[:, i] for i in range(CIC)]
                kps = [pk.tile([CQ, 450], F32, name="kp", tag="kp",
                               padded_shape=[P, 450]) for _ in range(8)]
                # one dynamic DMA pulls my batch's pre-act for all 4 chunks
                nc.sync.dma_start(
                    fh4[:],
                    p1r[:, :, ds(boff, NPIX)].rearrange("c p n -> p c n"))
                # f2's slice staged through DRAM (frees SP registers)
                nc.sync.dma_start(pmy2[:], p2r[:, :, ds(boff, NPIX)])
                for cic in range(CIC):
                    r32 = rp2.tile([P, NPIX], F32, name="r32", tag="r32", bufs=2)
                    nc.scalar.activation(
                        r32[:], f1h[cic][:], AF.Relu,
                        bias=bn0s[:, 1, cic: cic + 1],
                        scale=bn0s[:, 0, cic: cic + 1],
                    )
                    nc.sync.dma_start(f1g[cic, :, H: H + NPIX], r32[:])
                    nc.vector.tensor_copy(f1h[cic][:], r32[:])
                    for nt in range(8):
                        nc.tensor.matmul(
                            kps[nt], wkt[cic][:], r32[:, nt * 450: (nt + 1) * 450],
                            start=cic == 0, stop=cic == CIC - 1,
                        )
                for nt in range(8):
                    nc.vector.tensor_scalar_add(
                        ka[0:CQ, nt * 450: (nt + 1) * 450], kps[nt], bk[:]
                    )

                # f2 = bn_relu(reduced partials), written to padded map
                for cic in range(CIC):
                    rb2 = rp2.tile([P, NPIX], BF16, name="rb2", tag="rb2")
                    nc.sync.dma_start(rb2[:], pmy2[cic])
                    r32 = rp2.tile([P, NPIX], F32, name="r32", tag="r32", bufs=2)
                    nc.scalar.activation(
                        r32[:], rb2[:], AF.Relu,
                        bias=bn0c[:, 1, cic: cic + 1],
                        scale=bn0c[:, 0, cic: cic + 1],
                    )
                    nc.sync.dma_start(f2g[cic, :, H: H + NPIX], r32[:])

                # per-core windows (rows 15s-1 .. 15s+16 incl. halo);
                # single dynamic DMA per map (SP registers are scarce)
                nc.any.memset(f1w4[:], 0.0)
                nc.any.memset(f2w4[:], 0.0)
                nc.sync.dma_start(
                    f1w4[:, :, 0:WIN],
                    f1g[:, :, ds(woff, WIN)].rearrange("c p w -> p c w"))
                nc.sync.dma_start(
                    f2w4[:, :, 0:WIN],
                    f2g[:, :, ds(woff, WIN)].rearrange("c p w -> p c w"))

                # v = wv @ f1 (bf16), then transpose; before q (v only needs fh4)
                for cot in range(CIC):
                    vsb = vsp.tile([P, MP], BF16, name="vsb", tag="vsb")
                    nc.any.memset(vsb[:, NPIX:MP], 0.0)
                    for nt in range(8):
                        vp = pk.tile([P, 450], F32, name="vp", tag="kp")
                        for cic in range(CIC):
                            nc.tensor.matmul(
                                vp,
                                wvt[cic][:, cot * P: (cot + 1) * P],
                                f1h[cic][:, nt * 450: (nt + 1) * 450],
                                start=cic == 0, stop=cic == CIC - 1,
                            )
                        nc.vector.tensor_scalar_add(
                            vsb[:, nt * 450: (nt + 1) * 450], vp, bv[:, cot: cot + 1]
                        )
                    nc.sync.dma_start_transpose(vt[cot][:], vsb[:])

                # q from the f32 window
                for hf in range(2):
                    qp = pk.tile([CQ, 512], F32, name="qp", tag="kp",
                                 padded_shape=[P, 512])
                    for cic in range(CIC):
                        nc.tensor.matmul(
                            qp, wqt[cic][:], f1win[cic][:, hf * 512: (hf + 1) * 512],
                            start=cic == 0, stop=cic == CIC - 1,
                        )
                    nc.vector.tensor_scalar_add(
                        qa[0:CQ, hf * 512: (hf + 1) * 512], qp, bq[:]
                    )

            # hi/lo packing for the energy matmul (own pool, opened after c2
            # closes so it doesn't inflate phase-2's concurrent SBUF footprint):
            #   mm1: lhsT=[kh(64); aug(2); 0] rhs=[qh(64); augq(2); 0]
            #   mm2: lhsT=[kl(64); kh(64)]    rhs=[qh(64); ql(64)]
            hilo = ctx.enter_context(tc.tile_pool(name="hilo", bufs=1))
            kah = hilo.tile([P, MP], BF16, name="kah")
            kal = hilo.tile([P, MP], BF16, name="kal")
            qah = hilo.tile([P, WINP], BF16, name="qah")
            qal = hilo.tile([P, WINP], BF16, name="qal")
            nc.vector.memset(kah[:], 0.0)
            nc.vector.memset(qah[:], 0.0)
            nc.vector.tensor_copy(kah[0:66, :], ka[0:66, :])
            nc.vector.tensor_sub(kal[0:64, :], ka[0:64, :], kah[0:64, :])
            nc.vector.tensor_copy(kal[64:128, :], kah[0:64, :])
            nc.vector.tensor_copy(qah[0:66, :], qa[0:66, :])
            nc.vector.tensor_sub(qal[64:128, :], qa[0:64, :], qah[0:64, :])
            nc.vector.tensor_copy(qal[0:64, :], qah[0:64, :])

            # ================= phase 4a: CAM gram matrix (overlaps AR) ===========
            xfwin = [pers.tile([P, WINP], BF16, name=f"xfwin{i}") for i in range(CIC)]
            cen_sb = [mid.tile([P, CI], F32, name=f"cen{i}") for i in range(CIC)]
            with ExitStack() as c4:
                sb4 = c4.enter_context(tc.tile_pool(name="sb4", bufs=1))
                pc = c4.enter_context(tc.tile_pool(name="pc", bufs=2, space="PSUM"))
                xfh = sb4.tile([P, CIC, WINP], BF16, name="xfh")
                xfl = sb4.tile([P, CIC, WINP], BF16, name="xfl")
                xth = sb4.tile([P, 8, CIC, P], BF16, name="xth")
                xtl = sb4.tile([P, 8, CIC, P], BF16, name="xtl")
                tmpf = sb4.tile([P, 900], F32, name="tmpf")
                for i in range(CIC):
                    nc.any.memset(xfwin[i][:], 0.0)
                    nc.vector.tensor_copy(xfwin[i][:, 0:WIN], f2win[i][:, 0:WIN])
                    nc.any.memset(xfh[:, i, 900:WINP], 0.0)
                    nc.any.memset(xfl[:, i, 900:WINP], 0.0)
                    # hi/lo split of my 900 pixels (window cols 60:960)
                    nc.vector.tensor_copy(xfh[:, i, 0:900], f2win[i][:, 60:960])
                    nc.vector.tensor_copy(tmpf[:], xfh[:, i, 0:900])
                    nc.vector.tensor_sub(xfl[:, i, 0:900], f2win[i][:, 60:960], tmpf[:])
                    nc.sync.dma_start_transpose(xth[:, :, i, :], xfh[:, i, :])
                    nc.sync.dma_start_transpose(xtl[:, :, i, :], xfl[:, i, :])
                for ct in range(CIC):
                    cp = pc.tile([P, CI], F32, name="cp", tag="cp")
                    n_mm = 0
                    for nch in range(8):
                        for lh, rh in ((xth, xth), (xth, xtl), (xtl, xth)):
                            nc.tensor.matmul(
                                cp, lh[:, nch, ct, :],
                                rh[:, nch, :, :].rearrange("p a b -> p (a b)"),
                                start=n_mm == 0, stop=n_mm == 23,
                            )
                            n_mm += 1
                    nc.scalar.activation(cen_sb[ct][:], cp[:], AF.Copy)
                    nc.sync.dma_start(cen_in[ct], cen_sb[ct][:])
                if not _SKIP_CC:
                    nc.gpsimd.collective_compute(
                        "AllReduce", ALU.add,
                        replica_groups=GROUPS,
                        ins=[cen_in.opt()], outs=[cen_out.opt()],
                    )
                else:
                    nc.sync.dma_start(cen_out[:], cen_in[:])

            # ======================= phase 3: position attention =================
            sa_win = [mid.tile([P, WINP], BF16, name=f"sawin{i}") for i in range(CIC)]
            with ExitStack() as c3:
                sb3 = c3.enter_context(tc.tile_pool(name="sb3", bufs=1))
                ap3 = c3.enter_context(tc.tile_pool(name="ap3", bufs=3))
                pe3 = c3.enter_context(tc.tile_pool(name="pe3", bufs=2, space="PSUM"))
                psa = c3.enter_context(tc.tile_pool(name="psa", bufs=4, space="PSUM"))
                psum3 = c3.enter_context(tc.tile_pool(name="psum3", bufs=2, space="PSUM"))

                ones = sb3.tile([P, 1], BF16, name="ones")
                nc.any.memset(ones[:], 1.0)
                nshift = sb3.tile([P, 1], F32, name="nshift")
                nc.any.memset(nshift[:], -CSH)
                for hf in range(2):
                    hsl = slice(hf * 512, (hf + 1) * 512)
                    saps = [
                        psa.tile([P, 512], F32, name="sap", tag="sap")
                        for _ in range(CIC)
                    ]
                    sums = psum3.tile([1, 512], F32, name="sums", tag="sums",
                                      padded_shape=[P, 512])
                    for mc in range(MCH):
                        ep = pe3.tile([P, 512], F32, name="ep", tag="ep")
                        nc.tensor.matmul(
                            ep, kah[:, mc * P: (mc + 1) * P], qah[:, hsl],
                            start=True, stop=False,
                        )
                        nc.tensor.matmul(
                            ep, kal[:, mc * P: (mc + 1) * P], qal[:, hsl],
                            start=False, stop=True,
                        )
                        at = ap3.tile([P, 512], BF16, name="at", tag="at")
                        nc.scalar.activation(at[:], ep[:], AF.Exp,
                                             bias=nshift[:], scale=1.0)
                        nc.tensor.matmul(
                            sums, ones[:], at[:], start=mc == 0, stop=mc == MCH - 1
                        )
                        for cot in range(CIC):
                            nc.tensor.matmul(
                                saps[cot], vt[cot][:, mc, :], at[:],
                                start=mc == 0, stop=mc == MCH - 1,
                            )
                    ssb = sb3.tile([1, 512], F32, name="ssb", tag="ssb",
                                   padded_shape=[P, 512])
                    nc.scalar.activation(ssb[:], sums[:], AF.Copy)
                    rec = sb3.tile([1, 512], F32, name="rec", tag="rec",
                                   padded_shape=[P, 512])
                    nc.vector.reciprocal(rec[:], ssb[:])
                    nc.vector.tensor_mul(rec[:], rec[:], qmask[:, hsl])
                    rbp = pe3.tile([P, 512], F32, name="rbp", tag="ep")
                    nc.tensor.matmul(rbp, gsa[:], rec[:], start=True, stop=True)
                    recb = sb3.tile([P, 512], F32, name="recb", tag="recb")
                    nc.scalar.activation(recb[:], rbp[:], AF.Copy)
                    for cot in range(CIC):
                        tmp3 = sb3.tile([P, 512], F32, name="tmp3", tag="tmp3")
                        nc.vector.tensor_mul(tmp3[:], saps[cot][:], recb[:])
                        nc.vector.tensor_add(
                            sa_win[cot][:, hsl], tmp3[:], f1win[cot][:, hsl]
                        )

            # =================== phase 4b: CAM softmax + attention ===============
            sc_win = [mid.tile([P, WINP], BF16, name=f"scwin{i}") for i in range(CIC)]
            with ExitStack() as c4b:
                sb4b = c4b.enter_context(tc.tile_pool(name="sb4b", bufs=1))
                pc2 = c4b.enter_context(tc.tile_pool(name="pc2", bufs=2, space="PSUM"))
                cattT = sb4b.tile([P, CIC, CIC, P], BF16, name="cattT")
                crec = sb4b.tile([P, CIC], F32, name="crec")
                for ct in range(CIC):
                    cg = cen_sb[ct]
                    nc.sync.dma_start(cg[:], cen_out[ct])
                    rmin = sb4b.tile([P, 1], F32, name="rmin", tag="rmin")
                    nc.vector.tensor_reduce(rmin[:], cg[:], axis=AX.X, op=ALU.min)
                    cat = sb4b.tile([P, CI], BF16, name="cat", tag="cat", bufs=2)
                    csum = sb4b.tile([P, 1], F32, name="csum", tag="csum", bufs=2)
                    nc.scalar.activation(
                        cat[:], cg[:], AF.Exp, bias=rmin[:], scale=-1.0,
                        accum_out=csum[:],
                    )
                    nc.vector.reciprocal(crec[:, ct: ct + 1], csum[:])
                    nc.vector.tensor_mul(crec[:, ct: ct + 1], crec[:, ct: ct + 1],
                                         gsc[:])
                    nc.sync.dma_start_transpose(cattT[:, :, ct, :], cat[:])
                for ct in range(CIC):
                    for hf in range(2):
                        hsl = slice(hf * 512, (hf + 1) * 512)
                        scp = pc2.tile([P, 512], F32, name="scp", tag="scp")
                        for dch in range(CIC):
                            nc.tensor.matmul(
                                scp, cattT[:, dch, ct, :], xfwin[dch][:, hsl],
                                start=dch == 0, stop=dch == CIC - 1,
                            )
                        tmp4 = sb4b.tile([P, 512], F32, name="tmp4", tag="tmp4")
                        nc.scalar.activation(tmp4[:], scp[:], AF.Copy,
                                             scale=crec[:, ct: ct + 1])
                        nc.vector.tensor_add(
                            sc_win[ct][:, hsl], tmp4[:], f2win[ct][:, hsl]
                        )

            # ============= phase 5: pads, stage-2 convs, output heads ============
            late = ctx.enter_context(tc.tile_pool(name="late", bufs=1))
            sa_pad = [late.tile([P, 17, HP], BF16, name=f"sapad{i}") for i in range(CIC)]
            sc_pad = [late.tile([P, 17, HP], BF16, name=f"scpad{i}") for i in range(CIC)]
            for i in range(CIC):
                nc.any.memset(sa_pad[i][:], 0.0)
                nc.any.memset(sc_pad[i][:], 0.0)
                nc.vector.tensor_copy(
                    sa_pad[i][:, :, 1:61],
                    sa_win[i][:, 0:WIN].rearrange("p (r c) -> p r c", c=H),
                )
                nc.vector.tensor_copy(
                    sc_pad[i][:, :, 1:61],
                    sc_win[i][:, 0:WIN].rearrange("p (r c) -> p r c", c=H),
                )

            sa_conv = [late.tile([P, 900], BF16, name=f"sacv{i}") for i in range(CIC)]
            sc_conv = [late.tile([P, 900], BF16, name=f"sccv{i}") for i in range(CIC)]
            fsum = [late.tile([P, 900], BF16, name=f"fsum{i}") for i in range(CIC)]

            with ExitStack() as c5:
                sb5 = c5.enter_context(tc.tile_pool(name="sb5", bufs=1))
                wp5 = c5.enter_context(tc.tile_pool(name="wp5", bufs=4))
                pp5 = c5.enter_context(tc.tile_pool(name="pp5", bufs=3, space="PSUM"))
                ph5 = c5.enter_context(tc.tile_pool(name="ph5", bufs=2, space="PSUM"))

                bn1 = sb5.tile([P, 2, 2, CIC], F32, name="bn1")
                nc.sync.dma_start(bn1[:, 0], BN1S[:])
                nc.sync.dma_start(bn1[:, 1], BN1C[:])

                for bi, (wsrc, pad, cv) in enumerate(
                    ((w1sg, sa_pad, sa_conv), (w1cg, sc_pad, sc_conv))
                ):
                    for cot in range(CIC):
                        cps = [
                            pp5.tile([P, 300], F32, name="cp5", tag="cp5")
                            for _ in range(3)
                        ]
                        for cic in range(CIC):
                            wt9 = wp5.tile([P, 9, P], BF16, name="w1t", tag="w1t")
                            nc.sync.dma_start(
                                wt9[:],
                                wsrc[:, cic, :, cot * P: (cot + 1) * P]
                                .rearrange("o p q -> p o q"))
                            for off in range(9):
                                ky, kx = off // 3, off % 3
                                start = cic == 0 and off == 0
                                stop = cic == CIC - 1 and off == 8
                                for rt in range(3):
                                    rhs = pad[cic][
                                        :, rt * 5 + ky: rt * 5 + ky + 5, kx: kx + H
                                    ]
                                    nc.tensor.matmul(
                                        cps[rt], wt9[:, off, :], rhs,
                                        start=start, stop=stop
                                    )
                        for rt in range(3):
                            nc.scalar.activation(
                                cv[cot][:, rt * 300: (rt + 1) * 300], cps[rt][:],
                                AF.Relu, bias=bn1[:, bi, 1, cot: cot + 1],
                                scale=bn1[:, bi, 0, cot: cot + 1],
                            )
                for i in range(CIC):
                    nc.vector.tensor_add(fsum[i][:], sa_conv[i][:], sc_conv[i][:])

                w6 = sb5.tile([P, 3, CIC, CO], BF16, name="w6")
                b6 = sb5.tile([CO, 3], F32, name="b6", padded_shape=[P, 3])
                for j in range(3):
                    for cic in range(CIC):
                        nc.sync.dma_start(w6[:, j, cic, :], w678g[j, cic])
                for j, bsrc in enumerate((B8, B6, B7)):
                    nc.sync.dma_start(b6[:, j: j + 1], bsrc[:])
                for oi, src in enumerate((fsum, sa_conv, sc_conv)):
                    for hf in range(2):
                        hp = ph5.tile([CO, 450], F32, name="hp", tag="hp",
                                      padded_shape=[P, 450])
                        for cic in range(CIC):
                            nc.tensor.matmul(
                                hp, w6[:, oi, cic, :],
                                src[cic][:, hf * 450: (hf + 1) * 450],
                                start=cic == 0, stop=cic == CIC - 1,
                            )
                        osb = sb5.tile([CO, 450], BF16, name="osb", tag="osb",
                                       padded_shape=[P, 450])
                        nc.vector.tensor_scalar_add(osb[:], hp[:], b6[:, oi: oi + 1])
                        nc.sync.dma_start(OUT[oi, :, hf * 450: (hf + 1) * 450], osb[:])
            ctx.close()

    if split:
        _split_waits(nc)
    return nc


# ------------------------------------------------------------- host side ---

def _bn_fold(p):
    s, b, m, v = np.asarray(p, np.float32)
    a = s / np.sqrt(v + EPS)
    return a.astype(np.float32), (b - m * a).astype(np.float32)


def _bn_layout(a, b):
    # [P, 2, CIC]: [:, 0, c] = a-slice c, [:, 1, c] = b-slice c
    st = np.stack([a.reshape(CIC, P), b.reshape(CIC, P)])   # [2, CIC, P]
    return np.ascontiguousarray(st.transpose(2, 0, 1).astype(np.float32))


def host_prep(inputs):
    """Build the 8 per-core input maps."""
    inp = {k: np.asarray(v) for k, v in inputs.items()}
    x = inp["x"].astype(np.float32)
    d = inp["d"].astype(np.float32)
    lam = np.float32(inp["lamb"])
    B = x.shape[0]

    def w0_blocks(w):
        # [O=512, I=2048, 3, 3] -> per 256-ch slice c: [2, P, 9, CI] lhsT
        out = []
        for c in range(8):
            ws = w[:, c * 256:(c + 1) * 256]          # [512, 256, 3, 3]
            t = np.transpose(ws, (1, 2, 3, 0))        # [I256, 3, 3, O]
            out.append(np.ascontiguousarray(
                t.reshape(2, P, 9, CI).astype(bf)))
        return out

    def conv_w_full(w):
        # [512, 512, 3, 3] -> [9, 4, 128, 512]
        t = np.transpose(w, (2, 3, 1, 0))             # [3,3,512,512]
        return np.ascontiguousarray(t.reshape(9, CIC, P, CI).astype(bf))

    blk_s = w0_blocks(inp["w_s0"])
    blk_c = w0_blocks(inp["w_c0"])

    a0s, b0s = _bn_fold(inp["bn_s0"])
    a0c, b0c = _bn_fold(inp["bn_c0"])
    a1s, b1s = _bn_fold(inp["bn_s1"])
    a1c, b1c = _bn_fold(inp["bn_c1"])
    bn0s = _bn_layout(a0s, b0s)
    bn0c = _bn_layout(a0c, b0c)
    bn1s = _bn_layout(a1s, b1s)
    bn1c = _bn_layout(a1c, b1c)

    wqt = inp["wq"].T.reshape(CIC, P, CQ).astype(np.float32)
    wkt = inp["wk"].T.reshape(CIC, P, CQ).astype(np.float32)
    wqk8 = np.ascontiguousarray(np.stack([wqt, wkt])).reshape(8, QCH)
    wvt8 = np.ascontiguousarray(
        inp["wv"].T.reshape(CIC, P, CI).astype(bf)).reshape(8, VCH)
    w1s8 = conv_w_full(inp["w_s1"]).reshape(8, WCH)
    w1c8 = conv_w_full(inp["w_c1"]).reshape(8, WCH)
    w6t = inp["w6"].T.reshape(CIC, P, CO).astype(bf)
    w7t = inp["w7"].T.reshape(CIC, P, CO).astype(bf)
    w8t = inp["w8"].T.reshape(CIC, P, CO).astype(bf)
    w678_8 = np.ascontiguousarray(np.stack([w8t, w6t, w7t])).reshape(8, OCH)

    gsa = np.full((1, P), np.float32(inp["gamma_sa"]), np.float32)
    gsc = np.full((P, 1), np.float32(inp["gamma_sc"]), np.float32)

    in_maps = []
    for c in range(8):
        b_, s = c // 4, c % 4
        df = d[b_, 0].reshape(NPIX)
        dka = np.zeros((2, MP), np.float32)
        dka[0, :NPIX] = lam * df * df
        dka[0, NPIX:] = -1000.0
        dka[1, :NPIX] = df

        out_r0 = 15 * s
        dqa = np.zeros((2, WINP), np.float32)
        qmask = np.zeros((1, WINP), np.float32)
        dqa[0, :WIN] = 1.0
        for v_ in range(17):
            rv = out_r0 - 1 + v_
            if 0 <= rv < H:
                dqa[1, v_ * H:(v_ + 1) * H] = -2.0 * lam * d[b_, 0, rv]
                qmask[0, v_ * H:(v_ + 1) * H] = 1.0

        in_maps.append({
            "XP": np.ascontiguousarray(
                x[:, c * 256:(c + 1) * 256].reshape(2, 2, P, NPIX).astype(bf)),
            "W0SH": blk_s[c],
            "W0CH": blk_c[c],
            "BN0S": bn0s, "BN0C": bn0c,
            "W1SP": w1s8[c], "W1CP": w1c8[c],
            "WVTP": wvt8[c], "W678P": w678_8[c], "WQKP": wqk8[c],
            "BQ": inp["bq"].reshape(CQ, 1).astype(np.float32),
            "BK": inp["bk"].reshape(CQ, 1).astype(np.float32),
            "BV": np.ascontiguousarray(
                inp["bv"].reshape(CIC, P).T.astype(np.float32)),
            "DKA": dka, "DQA": dqa, "QMASK": qmask,
            "GSA": gsa, "GSC": gsc,
            "BN1S": bn1s, "BN1C": bn1c,
            "B6": inp["b6"].reshape(CO, 1).astype(np.float32),
            "B7": inp["b7"].reshape(CO, 1).astype(np.float32),
            "B8": inp["b8"].reshape(CO, 1).astype(np.float32),
        })
    return in_maps


def assemble(results):
    """results: list of 8 dicts with 'OUT' [3, 40, 900] -> output tuple."""
    outs = []
    for b_ in range(2):
        rows = [np.asarray(results[4 * b_ + s]["OUT"]).astype(np.float32).reshape(
            3, CO, 15, H) for s in range(4)]
        outs.append(np.concatenate(rows, axis=2))        # [3, 40, 60, 60]
    full = np.stack(outs, axis=1)                        # [3, B, 40, 60, 60]
    return full[0], full[1], full[2]


def _split_waits(nc, keep=1):
    """Walrus in this container accepts at most one embedded sync-wait per
    instruction; Tile emits several. Turn extra waits into standalone
    single-wait EventSemaphore instructions before the owner, same engine."""
    n_split = 0
    for fn in nc.m.functions:
        for bb in fn.blocks:
            new_insts = []
            for inst in bb.instructions:
                si = inst.sync_info
                if si is not None and len(si.on_wait) > keep:
                    waits = list(si.on_wait)
                    head, tail = waits[:-keep], waits[-keep:]
                    for j, w in enumerate(head):
                        new_insts.append(mybir.InstEventSemaphore(
                            name=f"{inst.name}-ws{j}",
                            engine=inst.engine,
                            ins=[], outs=[],
                            sync_info=mybir.SyncInfo(on_wait=[w], on_update=[]),
                        ))
                        n_split += 1
                    inst.sync_info = mybir.SyncInfo(
                        on_wait=tail, on_update=list(si.on_update))
                new_insts.append(inst)
            bb.instructions.clear()
            bb.instructions.extend(new_insts)
    return n_split


_NC = None


def kernel(**inputs):
    global _NC
    if _NC is None:
        _NC = build_nc()
    from concourse.bass_utils import run_bass_kernel_spmd
    in_maps = host_prep(inputs)
    res = run_bass_kernel_spmd(_NC, in_maps, core_ids=list(range(8)))
    return assemble(res.results)


# revision 37
# speedup vs baseline: 1.1508x; 1.0011x over previous
"""DANetHead (dual attention head) Trainium2 kernel.

Strategy (8 NeuronCores): 2-way data parallel over batch B=2 (core groups
[0-3], [4-7]) x 4-way model parallel within each batch group.

Host->device traffic is minimized (it dominates the graded wall time):
  - x is channel-split: each core receives 512 of 2048 input channels
    (unreplicated). Stage-1 3x3 convs compute partial sums over those
    channels for ALL 512 output channels; an AllReduce over the batch
    group forms the true pre-activation, and bn_relu is applied locally.
  - Stage-1 conv weights: cores c and c+4 need identical blocks, so each
    receives half and an AllGather over pairs [[0,4],...] reconstructs.
  - Stage-2 conv weights + attention weights (identical on all 8 cores)
    are shipped 1/8th per core and AllGathered over all 8.

Matmuls run in bf16 (f32 PSUM accumulation) except the attention/CAM logits
which use f32 / hi-lo bf16 splitting to keep softmax inputs accurate.
"""

import os
from contextlib import ExitStack

import numpy as np
import ml_dtypes

import concourse.bass as bass
import concourse.tile as tile
import concourse.mybir as mybir
from concourse.bass import ds

dt = mybir.dt
F32 = dt.float32
BF16 = dt.bfloat16
AF = mybir.ActivationFunctionType
AX = mybir.AxisListType
ALU = mybir.AluOpType

P = 128
H = 60
HP = 62
NPIX = 3600          # 60*60
NPAD = 3720          # 60 zero + 3600 + 60 zero (padded full feature map)
MP = 3712            # 29*128, padded key/value pixel count
MCH = 29             # m chunks
WIN = 1020           # 17 rows * 60 query window
WINP = 1024          # padded window
CIN = 2048
CI = 512
CIC = 4              # 512 / 128
CQ = 64
CO = 40
CSH = 25.0           # softmax shift constant (max logit ~24.8)
GROUPS = [[0, 1, 2, 3], [4, 5, 6, 7]]
PAIRS = [[0, 4], [1, 5], [2, 6], [3, 7]]
WCH = 9 * CIC * P * CI // 8      # 294912  (1/8 of a stage-2 conv weight)
VCH = CIC * P * CI // 8          # 32768
OCH = 3 * CIC * P * CO // 8      # 7680
QCH = 2 * CIC * P * CQ // 8      # 8192
EPS = 1e-5

bf = ml_dtypes.bfloat16
_SKIP_CC = bool(int(os.environ.get("DANET_SKIP_CC", "0")))


# ---------------------------------------------------------------- builder ---

def build_nc(split=True, reps=1):
    nc = bass.Bass(num_devices=8)

    # ---- inputs (per-core contents differ; shapes identical) ----
    # XP: this core's 256 input channels for BOTH batches [batch, cic2, P, pix]
    XP = nc.dram_tensor("XP", [2, 2, P, NPIX], BF16, kind="ExternalInput")
    # W0*H: weights for those 256 channels -> all 512 outs (batch-independent,
    # so they serve both batches locally; no weight exchange needed)
    W0SH = nc.dram_tensor("W0SH", [2, P, 9, CI], BF16, kind="ExternalInput")
    W0CH = nc.dram_tensor("W0CH", [2, P, 9, CI], BF16, kind="ExternalInput")
    BN0S = nc.dram_tensor("BN0S", [P, 2, CIC], F32, kind="ExternalInput")
    BN0C = nc.dram_tensor("BN0C", [P, 2, CIC], F32, kind="ExternalInput")
    W1SP = nc.dram_tensor("W1SP", [WCH], BF16, kind="ExternalInput")
    W1CP = nc.dram_tensor("W1CP", [WCH], BF16, kind="ExternalInput")
    WVTP = nc.dram_tensor("WVTP", [VCH], BF16, kind="ExternalInput")
    W678P = nc.dram_tensor("W678P", [OCH], BF16, kind="ExternalInput")
    WQKP = nc.dram_tensor("WQKP", [QCH], F32, kind="ExternalInput")
    BQ = nc.dram_tensor("BQ", [CQ, 1], F32, kind="ExternalInput")
    BK = nc.dram_tensor("BK", [CQ, 1], F32, kind="ExternalInput")
    BV = nc.dram_tensor("BV", [P, CIC], F32, kind="ExternalInput")
    DKA = nc.dram_tensor("DKA", [2, MP], F32, kind="ExternalInput")
    DQA = nc.dram_tensor("DQA", [2, WINP], F32, kind="ExternalInput")
    QMASK = nc.dram_tensor("QMASK", [1, WINP], F32, kind="ExternalInput")
    GSA = nc.dram_tensor("GSA", [1, P], F32, kind="ExternalInput")
    GSC = nc.dram_tensor("GSC", [P, 1], F32, kind="ExternalInput")
    BN1S = nc.dram_tensor("BN1S", [P, 2, CIC], F32, kind="ExternalInput")
    BN1C = nc.dram_tensor("BN1C", [P, 2, CIC], F32, kind="ExternalInput")
    B6 = nc.dram_tensor("B6", [CO, 1], F32, kind="ExternalInput")
    B7 = nc.dram_tensor("B7", [CO, 1], F32, kind="ExternalInput")
    B8 = nc.dram_tensor("B8", [CO, 1], F32, kind="ExternalInput")
    OUT = nc.dram_tensor("OUT", [3, CO, 900], BF16, kind="ExternalOutput")

    with tile.TileContext(nc) as tc:
        for _rep in range(reps):
            ctx = ExitStack()
            dram = ctx.enter_context(tc.tile_pool(name="dram", bufs=1, space="DRAM"))

            # window start within the padded full features: 900 * (core % 4)
            woff = (nc.sync.partition_id() % 4) * 900
            # my batch's offset within the two-batch reduced partials
            boff = (nc.sync.partition_id() // 4) * NPIX

            # gathered weights
            wqkg = dram.tile([2, CIC, P, CQ], F32, name="wqkg")
            wvg = dram.tile([CIC, P, CI], BF16, name="wvg")
            w1sg = dram.tile([9, CIC, P, CI], BF16, name="w1sg")
            w1cg = dram.tile([9, CIC, P, CI], BF16, name="w1cg")
            w678g = dram.tile([3, CIC, P, CO], BF16, name="w678g")

            # collectives cannot read IO tensors: stage inputs in DRAM tiles
            ALL8 = [[0, 1, 2, 3, 4, 5, 6, 7]]

            def ag(groups, src, shape, dtype, out_tile, name):
                st = dram.tile(shape, dtype, name=name)
                nc.sync.dma_start(st[:], src[:])
                nc.gpsimd.collective_compute(
                    "AllGather", ALU.bypass, replica_groups=groups,
                    ins=[st.opt()], outs=[out_tile.opt()],
                )

            ag(ALL8, WQKP, [QCH], F32, wqkg, "wqkp_st")
            ag(ALL8, WVTP, [VCH], BF16, wvg, "wvtp_st")
            ag(ALL8, W1SP, [WCH], BF16, w1sg, "w1sp_st")
            ag(ALL8, W1CP, [WCH], BF16, w1cg, "w1cp_st")
            ag(ALL8, W678P, [OCH], BF16, w678g, "w678p_st")

            # partial conv pre-activations (both batches) and reduced versions
            p1 = dram.tile([CIC, P, 2 * NPIX], BF16, name="p1")
            p2 = dram.tile([CIC, P, 2 * NPIX], BF16, name="p2")
            p1r = dram.tile([CIC, P, 2 * NPIX], BF16, name="p1r")
            p2r = dram.tile([CIC, P, 2 * NPIX], BF16, name="p2r")
            f1g = dram.tile([CIC, P, NPAD], F32, name="f1g")
            f2g = dram.tile([CIC, P, NPAD], F32, name="f2g")
            cen_in = dram.tile([CIC, P, CI], F32, name="cen_in")
            cen_out = dram.tile([CIC, P, CI], F32, name="cen_out")

            # =========== stage 1: partial 3x3 convs (512 ins -> 512 outs) =======
            with ExitStack() as c1:
                sb1 = c1.enter_context(tc.tile_pool(name="sb1", bufs=1))
                fb1 = c1.enter_context(tc.tile_pool(name="fb1", bufs=2))
                pp1 = c1.enter_context(tc.tile_pool(name="pp1", bufs=8, space="PSUM"))

                zsb = sb1.tile([P, H], F32, name="zsb")
                nc.any.memset(zsb[:], 0.0)
                for fg_ in (f1g, f2g):
                    for cic in range(CIC):
                        nc.sync.dma_start(fg_[cic, :, 0:H], zsb[:])
                        nc.sync.dma_start(fg_[cic, :, NPAD - H: NPAD], zsb[:])

                xpad = [[sb1.tile([P, HP * HP], BF16, name=f"xpad{b}{c2}")
                         for c2 in range(2)] for b in range(2)]
                for b in range(2):
                    for c2 in range(2):
                        nc.any.memset(xpad[b][c2][:], 0.0)
                        nc.sync.dma_start(
                            xpad[b][c2][:].rearrange(
                                "p (r c) -> p r c", c=HP)[:, 1:61, 1:61],
                            XP[b, c2].rearrange("p (r c) -> p r c", c=H),
                        )

                w0s_sb = [sb1.tile([P, 9, CI], BF16, name=f"w0s{i}")
                          for i in range(2)]
                w0c_sb = [sb1.tile([P, 9, CI], BF16, name=f"w0c{i}")
                          for i in range(2)]
                for c2 in range(2):
                    nc.sync.dma_start(w0s_sb[c2][:], W0SH[c2])
                    nc.sync.dma_start(w0c_sb[c2][:], W0CH[c2])

                for wsb, pdst, prd in ((w0s_sb, p1, p1r), (w0c_sb, p2, p2r)):
                    for b in range(2):
                        for ot in range(CIC):
                            pts = [
                                pp1.tile([P, 480], F32, name="s1p", tag="s1p")
                                for _ in range(8)
                            ]
                            for c2 in range(2):
                                xv = xpad[b][c2][:].rearrange(
                                    "p (r c) -> p r c", c=HP)
                                for off in range(9):
                                    ky, kx = off // 3, off % 3
                                    start = c2 == 0 and off == 0
                                    stop = c2 == 1 and off == 8
                                    for t in range(8):
                                        rows = 8 if t < 7 else 4
                                        rhs = xv[:, ky + 8 * t: ky + 8 * t + rows,
                                                 kx: kx + H]
                                        nc.tensor.matmul(
                                            pts[t][:, : rows * H],
                                            wsb[c2][:, off, ot * P: (ot + 1) * P],
                                            rhs, start=start, stop=stop,
                                        )
                            fbuf = fb1.tile([P, NPIX], BF16, name="fbuf", tag="fbuf")
                            for t in range(8):
                                rows = 8 if t < 7 else 4
                                nc.scalar.activation(
                                    fbuf[:, t * 480: t * 480 + rows * H],
                                    pts[t][:, : rows * H], AF.Copy,
                                )
                            nc.sync.dma_start(
                                pdst[ot, :, b * NPIX: (b + 1) * NPIX], fbuf[:])
                    nc.gpsimd.collective_compute(
                        "AllReduce", ALU.add, replica_groups=ALL8,
                        ins=[pdst.opt()], outs=[prd.opt()],
                    )

            # ====================== phase 2: bn_relu, windows, k, q, v ==========
            pers = ctx.enter_context(tc.tile_pool(name="pers", bufs=1))
            mid = ctx.enter_context(tc.tile_pool(name="mid", bufs=1))

            bn0s = pers.tile([P, 2, CIC], F32, name="bn0s")
            bn0c = pers.tile([P, 2, CIC], F32, name="bn0c")
            nc.sync.dma_start(bn0s[:], BN0S[:])
            nc.sync.dma_start(bn0c[:], BN0C[:])

            wqt = [pers.tile([P, CQ], F32, name=f"wqt{i}") for i in range(CIC)]
            wkt = [pers.tile([P, CQ], F32, name=f"wkt{i}") for i in range(CIC)]
            wvt = [pers.tile([P, CI], BF16, name=f"wvt{i}") for i in range(CIC)]
            for i in range(CIC):
                nc.sync.dma_start(wqt[i][:], wqkg[0, i])
                nc.sync.dma_start(wkt[i][:], wqkg[1, i])
                nc.sync.dma_start(wvt[i][:], wvg[i])
            bq = pers.tile([CQ, 1], F32, name="bq", padded_shape=[P, 1])
            bk = pers.tile([CQ, 1], F32, name="bk", padded_shape=[P, 1])
            bv = pers.tile([P, CIC], F32, name="bv")
            nc.sync.dma_start(bq[:], BQ[:])
            nc.sync.dma_start(bk[:], BK[:])
            nc.sync.dma_start(bv[:], BV[:])
            gsa = pers.tile([1, P], F32, name="gsa", padded_shape=[P, P])
            gsc = pers.tile([P, 1], F32, name="gsc")
            qmask = pers.tile([1, WINP], F32, name="qmask", padded_shape=[P, WINP])
            nc.sync.dma_start(gsa[:], GSA[:])
            nc.sync.dma_start(gsc[:], GSC[:])
            nc.sync.dma_start(qmask[:], QMASK[:])

            ka = mid.tile([P, MP], F32, name="ka")
            qa = mid.tile([P, WINP], F32, name="qa")
            nc.any.memset(ka[:], 0.0)
            nc.any.memset(qa[:], 0.0)
            nc.sync.dma_start(ka[64:66, :], DKA[:])
            nc.sync.dma_start(qa[64:66, :], DQA[:])

            f1w4 = pers.tile([P, CIC, WINP], F32, name="f1w4")
            f2w4 = pers.tile([P, CIC, WINP], F32, name="f2w4")
            f1win = [f1w4[:, i] for i in range(CIC)]
            f2win = [f2w4[:, i] for i in range(CIC)]
            vt = [pers.tile([P, MCH, P], BF16, name=f"vt{i}") for i in range(CIC)]
            pmy2 = dram.tile([CIC, P, NPIX], BF16, name="pmy2")

            with ExitStack() as c2:
                sb2 = c2.enter_context(tc.tile_pool(name="sb2", bufs=1))
                rp2 = c2.enter_context(tc.tile_pool(name="rp2", bufs=1))
                pk = c2.enter_context(tc.tile_pool(name="pk", bufs=8, space="PSUM"))

                vsp = c2.enter_context(tc.tile_pool(name="vsp", bufs=2))
                fh4 = sb2.tile([P, CIC, NPIX], BF16, name="fh4")
                f1h = [fh4[:, i] for i in range(CIC)]
                kps = [pk.tile([CQ, 450], F32, name="kp", tag="kp",
                               padded_shape=[P, 450]) for _ in range(8)]
                # one dynamic DMA pulls my batch's pre-act for all 4 chunks
                nc.sync.dma_start(
                    fh4[:],
                    p1r[:, :, ds(boff, NPIX)].rearrange("c p n -> p c n"))
                # f2's slice staged through DRAM (frees SP registers)
                nc.sync.dma_start(pmy2[:], p2r[:, :, ds(boff, NPIX)])
                for cic in range(CIC):
                    r32 = rp2.tile([P, NPIX], F32, name="r32", tag="r32")
                    nc.scalar.activation(
                        r32[:], f1h[cic][:], AF.Relu,
                        bias=bn0s[:, 1, cic: cic + 1],
                        scale=bn0s[:, 0, cic: cic + 1],
                    )
                    nc.sync.dma_start(f1g[cic, :, H: H + NPIX], r32[:])
                    nc.vector.tensor_copy(f1h[cic][:], r32[:])
                    for nt in range(8):
                        nc.tensor.matmul(
                            kps[nt], wkt[cic][:], r32[:, nt * 450: (nt + 1) * 450],
                            start=cic == 0, stop=cic == CIC - 1,
                        )
                for nt in range(8):
                    nc.vector.tensor_scalar_add(
                        ka[0:CQ, nt * 450: (nt + 1) * 450], kps[nt], bk[:]
                    )

                # f2 = bn_relu(reduced partials), written to padded map
                for cic in range(CIC):
                    rb2 = rp2.tile([P, NPIX], BF16, name="rb2", tag="rb2")
                    nc.sync.dma_start(rb2[:], pmy2[cic])
                    r32 = rp2.tile([P, NPIX], F32, name="r32", tag="r32")
                    nc.scalar.activation(
                        r32[:], rb2[:], AF.Relu,
                        bias=bn0c[:, 1, cic: cic + 1],
                        scale=bn0c[:, 0, cic: cic + 1],
                    )
                    nc.sync.dma_start(f2g[cic, :, H: H + NPIX], r32[:])

                # per-core windows (rows 15s-1 .. 15s+16 incl. halo);
                # single dynamic DMA per map (SP registers are scarce)
                nc.any.memset(f1w4[:], 0.0)
                nc.any.memset(f2w4[:], 0.0)
                nc.sync.dma_start(
                    f1w4[:, :, 0:WIN],
                    f1g[:, :, ds(woff, WIN)].rearrange("c p w -> p c w"))
                nc.sync.dma_start(
                    f2w4[:, :, 0:WIN],
                    f2g[:, :, ds(woff, WIN)].rearrange("c p w -> p c w"))

                # v = wv @ f1 (bf16), then transpose; before q (v only needs fh4)
                for cot in range(CIC):
                    vsb = vsp.tile([P, MP], BF16, name="vsb", tag="vsb")
                    nc.any.memset(vsb[:, NPIX:MP], 0.0)
                    for nt in range(8):
                        vp = pk.tile([P, 450], F32, name="vp", tag="kp")
                        for cic in range(CIC):
                            nc.tensor.matmul(
                                vp,
                                wvt[cic][:, cot * P: (cot + 1) * P],
                                f1h[cic][:, nt * 450: (nt + 1) * 450],
                                start=cic == 0, stop=cic == CIC - 1,
                            )
                        nc.vector.tensor_scalar_add(
                            vsb[:, nt * 450: (nt + 1) * 450], vp, bv[:, cot: cot + 1]
                        )
                    nc.sync.dma_start_transpose(vt[cot][:], vsb[:])

                # q from the f32 window
                for hf in range(2):
                    qp = pk.tile([CQ, 512], F32, name="qp", tag="kp",
                                 padded_shape=[P, 512])
                    for cic in range(CIC):
                        nc.tensor.matmul(
                            qp, wqt[cic][:], f1win[cic][:, hf * 512: (hf + 1) * 512],
                            start=cic == 0, stop=cic == CIC - 1,
                        )
                    nc.vector.tensor_scalar_add(
                        qa[0:CQ, hf * 512: (hf + 1) * 512], qp, bq[:]
                    )

            # hi/lo packing for the energy matmul (own pool, opened after c2
            # closes so it doesn't inflate phase-2's concurrent SBUF footprint):
            #   mm1: lhsT=[kh(64); aug(2); 0] rhs=[qh(64); augq(2); 0]
            #   mm2: lhsT=[kl(64); kh(64)]    rhs=[qh(64); ql(64)]
            hilo = ctx.enter_context(tc.tile_pool(name="hilo", bufs=1))
            kah = hilo.tile([P, MP], BF16, name="kah")
            kal = hilo.tile([P, MP], BF16, name="kal")
            qah = hilo.tile([P, WINP], BF16, name="qah")
            qal = hilo.tile([P, WINP], BF16, name="qal")
            nc.vector.memset(kah[:], 0.0)
            nc.vector.memset(qah[:], 0.0)
            nc.vector.tensor_copy(kah[0:66, :], ka[0:66, :])
            nc.vector.tensor_sub(kal[0:64, :], ka[0:64, :], kah[0:64, :])
            nc.vector.tensor_copy(kal[64:128, :], kah[0:64, :])
            nc.vector.tensor_copy(qah[0:66, :], qa[0:66, :])
            nc.vector.tensor_sub(qal[64:128, :], qa[0:64, :], qah[0:64, :])
            nc.vector.tensor_copy(qal[0:64, :], qah[0:64, :])

            # ================= phase 4a: CAM gram matrix (overlaps AR) ===========
            xfwin = [pers.tile([P, WINP], BF16, name=f"xfwin{i}") for i in range(CIC)]
            cen_sb = [mid.tile([P, CI], F32, name=f"cen{i}") for i in range(CIC)]
            with ExitStack() as c4:
                sb4 = c4.enter_context(tc.tile_pool(name="sb4", bufs=1))
                pc = c4.enter_context(tc.tile_pool(name="pc", bufs=2, space="PSUM"))
                xfh = sb4.tile([P, CIC, WINP], BF16, name="xfh")
                xfl = sb4.tile([P, CIC, WINP], BF16, name="xfl")
                xth = sb4.tile([P, 8, CIC, P], BF16, name="xth")
                xtl = sb4.tile([P, 8, CIC, P], BF16, name="xtl")
                tmpf = sb4.tile([P, 900], F32, name="tmpf")
                for i in range(CIC):
                    nc.any.memset(xfwin[i][:], 0.0)
                    nc.vector.tensor_copy(xfwin[i][:, 0:WIN], f2win[i][:, 0:WIN])
                    nc.any.memset(xfh[:, i, 900:WINP], 0.0)
                    nc.any.memset(xfl[:, i, 900:WINP], 0.0)
                    # hi/lo split of my 900 pixels (window cols 60:960)
                    nc.vector.tensor_copy(xfh[:, i, 0:900], f2win[i][:, 60:960])
                    nc.vector.tensor_copy(tmpf[:], xfh[:, i, 0:900])
                    nc.vector.tensor_sub(xfl[:, i, 0:900], f2win[i][:, 60:960], tmpf[:])
                    nc.sync.dma_start_transpose(xth[:, :, i, :], xfh[:, i, :])
                    nc.sync.dma_start_transpose(xtl[:, :, i, :], xfl[:, i, :])
                for ct in range(CIC):
                    cp = pc.tile([P, CI], F32, name="cp", tag="cp")
                    n_mm = 0
                    for nch in range(8):
                        for lh, rh in ((xth, xth), (xth, xtl), (xtl, xth)):
                            nc.tensor.matmul(
                                cp, lh[:, nch, ct, :],
                                rh[:, nch, :, :].rearrange("p a b -> p (a b)"),
                                start=n_mm == 0, stop=n_mm == 23,
                            )
                            n_mm += 1
                    nc.scalar.activation(cen_sb[ct][:], cp[:], AF.Copy)
                    nc.sync.dma_start(cen_in[ct], cen_sb[ct][:])
                if not _SKIP_CC:
                    nc.gpsimd.collective_compute(
                        "AllReduce", ALU.add,
                        replica_groups=GROUPS,
                        ins=[cen_in.opt()], outs=[cen_out.opt()],
                    )
                else:
                    nc.sync.dma_start(cen_out[:], cen_in[:])

            # ======================= phase 3: position attention =================
            sa_win = [mid.tile([P, WINP], BF16, name=f"sawin{i}") for i in range(CIC)]
            with ExitStack() as c3:
                sb3 = c3.enter_context(tc.tile_pool(name="sb3", bufs=1))
                ap3 = c3.enter_context(tc.tile_pool(name="ap3", bufs=3))
                pe3 = c3.enter_context(tc.tile_pool(name="pe3", bufs=2, space="PSUM"))
                psa = c3.enter_context(tc.tile_pool(name="psa", bufs=4, space="PSUM"))
                psum3 = c3.enter_context(tc.tile_pool(name="psum3", bufs=2, space="PSUM"))

                ones = sb3.tile([P, 1], BF16, name="ones")
                nc.any.memset(ones[:], 1.0)
                nshift = sb3.tile([P, 1], F32, name="nshift")
                nc.any.memset(nshift[:], -CSH)
                for hf in range(2):
                    hsl = slice(hf * 512, (hf + 1) * 512)
                    saps = [
                        psa.tile([P, 512], F32, name="sap", tag="sap")
                        for _ in range(CIC)
                    ]
                    sums = psum3.tile([1, 512], F32, name="sums", tag="sums",
                                      padded_shape=[P, 512])
                    for mc in range(MCH):
                        ep = pe3.tile([P, 512], F32, name="ep", tag="ep")
                        nc.tensor.matmul(
                            ep, kah[:, mc * P: (mc + 1) * P], qah[:, hsl],
                            start=True, stop=False,
                        )
                        nc.tensor.matmul(
                            ep, kal[:, mc * P: (mc + 1) * P], qal[:, hsl],
                            start=False, stop=True,
                        )
                        at = ap3.tile([P, 512], BF16, name="at", tag="at")
                        nc.scalar.activation(at[:], ep[:], AF.Exp,
                                             bias=nshift[:], scale=1.0)
                        nc.tensor.matmul(
                            sums, ones[:], at[:], start=mc == 0, stop=mc == MCH - 1
                        )
                        for cot in range(CIC):
                            nc.tensor.matmul(
                                saps[cot], vt[cot][:, mc, :], at[:],
                                start=mc == 0, stop=mc == MCH - 1,
                            )
                    ssb = sb3.tile([1, 512], F32, name="ssb", tag="ssb",
                                   padded_shape=[P, 512])
                    nc.scalar.activation(ssb[:], sums[:], AF.Copy)
                    rec = sb3.tile([1, 512], F32, name="rec", tag="rec",
                                   padded_shape=[P, 512])
                    nc.vector.reciprocal(rec[:], ssb[:])
                    nc.vector.tensor_mul(rec[:], rec[:], qmask[:, hsl])
                    rbp = pe3.tile([P, 512], F32, name="rbp", tag="ep")
                    nc.tensor.matmul(rbp, gsa[:], rec[:], start=True, stop=True)
                    recb = sb3.tile([P, 512], F32, name="recb", tag="recb")
                    nc.scalar.activation(recb[:], rbp[:], AF.Copy)
                    for cot in range(CIC):
                        tmp3 = sb3.tile([P, 512], F32, name="tmp3", tag="tmp3")
                        nc.vector.tensor_mul(tmp3[:], saps[cot][:], recb[:])
                        nc.vector.tensor_add(
                            sa_win[cot][:, hsl], tmp3[:], f1win[cot][:, hsl]
                        )

            # =================== phase 4b: CAM softmax + attention ===============
            sc_win = [mid.tile([P, WINP], BF16, name=f"scwin{i}") for i in range(CIC)]
            with ExitStack() as c4b:
                sb4b = c4b.enter_context(tc.tile_pool(name="sb4b", bufs=1))
                pc2 = c4b.enter_context(tc.tile_pool(name="pc2", bufs=2, space="PSUM"))
                cattT = sb4b.tile([P, CIC, CIC, P], BF16, name="cattT")
                crec = sb4b.tile([P, CIC], F32, name="crec")
                for ct in range(CIC):
                    cg = cen_sb[ct]
                    nc.sync.dma_start(cg[:], cen_out[ct])
                    rmin = sb4b.tile([P, 1], F32, name="rmin", tag="rmin")
                    nc.vector.tensor_reduce(rmin[:], cg[:], axis=AX.X, op=ALU.min)
                    cat = sb4b.tile([P, CI], BF16, name="cat", tag="cat", bufs=2)
                    csum = sb4b.tile([P, 1], F32, name="csum", tag="csum", bufs=2)
                    nc.scalar.activation(
                        cat[:], cg[:], AF.Exp, bias=rmin[:], scale=-1.0,
                        accum_out=csum[:],
                    )
                    nc.vector.reciprocal(crec[:, ct: ct + 1], csum[:])
                    nc.vector.tensor_mul(crec[:, ct: ct + 1], crec[:, ct: ct + 1],
                                         gsc[:])
                    nc.sync.dma_start_transpose(cattT[:, :, ct, :], cat[:])
                for ct in range(CIC):
                    for hf in range(2):
                        hsl = slice(hf * 512, (hf + 1) * 512)
                        scp = pc2.tile([P, 512], F32, name="scp", tag="scp")
                        for dch in range(CIC):
                            nc.tensor.matmul(
                                scp, cattT[:, dch, ct, :], xfwin[dch][:, hsl],
                                start=dch == 0, stop=dch == CIC - 1,
                            )
                        tmp4 = sb4b.tile([P, 512], F32, name="tmp4", tag="tmp4")
                        nc.scalar.activation(tmp4[:], scp[:], AF.Copy,
                                             scale=crec[:, ct: ct + 1])
                        nc.vector.tensor_add(
                            sc_win[ct][:, hsl], tmp4[:], f2win[ct][:, hsl]
                        )

            # ============= phase 5: pads, stage-2 convs, output heads ============
            late = ctx.enter_context(tc.tile_pool(name="late", bufs=1))
            sa_pad = [late.tile([P, 17, HP], BF16, name=f"sapad{i}") for i in range(CIC)]
            sc_pad = [late.tile([P, 17, HP], BF16, name=f"scpad{i}") for i in range(CIC)]
            for i in range(CIC):
                nc.any.memset(sa_pad[i][:], 0.0)
                nc.any.memset(sc_pad[i][:], 0.0)
                nc.vector.tensor_copy(
                    sa_pad[i][:, :, 1:61],
                    sa_win[i][:, 0:WIN].rearrange("p (r c) -> p r c", c=H),
                )
                nc.vector.tensor_copy(
                    sc_pad[i][:, :, 1:61],
                    sc_win[i][:, 0:WIN].rearrange("p (r c) -> p r c", c=H),
                )

            sa_conv = [late.tile([P, 900], BF16, name=f"sacv{i}") for i in range(CIC)]
            sc_conv = [late.tile([P, 900], BF16, name=f"sccv{i}") for i in range(CIC)]
            fsum = [late.tile([P, 900], BF16, name=f"fsum{i}") for i in range(CIC)]

            with ExitStack() as c5:
                sb5 = c5.enter_context(tc.tile_pool(name="sb5", bufs=1))
                wp5 = c5.enter_context(tc.tile_pool(name="wp5", bufs=4))
                pp5 = c5.enter_context(tc.tile_pool(name="pp5", bufs=3, space="PSUM"))
                ph5 = c5.enter_context(tc.tile_pool(name="ph5", bufs=2, space="PSUM"))

                bn1 = sb5.tile([P, 2, 2, CIC], F32, name="bn1")
                nc.sync.dma_start(bn1[:, 0], BN1S[:])
                nc.sync.dma_start(bn1[:, 1], BN1C[:])

                for bi, (wsrc, pad, cv) in enumerate(
                    ((w1sg, sa_pad, sa_conv), (w1cg, sc_pad, sc_conv))
                ):
                    for cot in range(CIC):
                        cps = [
                            pp5.tile([P, 300], F32, name="cp5", tag="cp5")
                            for _ in range(3)
                        ]
                        for cic in range(CIC):
                            wt9 = wp5.tile([P, 9, P], BF16, name="w1t", tag="w1t")
                            nc.sync.dma_start(
                                wt9[:],
                                wsrc[:, cic, :, cot * P: (cot + 1) * P]
                                .rearrange("o p q -> p o q"))
                            for off in range(9):
                                ky, kx = off // 3, off % 3
                                start = cic == 0 and off == 0
                                stop = cic == CIC - 1 and off == 8
                                for rt in range(3):
                                    rhs = pad[cic][
                                        :, rt * 5 + ky: rt * 5 + ky + 5, kx: kx + H
                                    ]
                                    nc.tensor.matmul(
                                        cps[rt], wt9[:, off, :], rhs,
                                        start=start, stop=stop
                                    )
                        for rt in range(3):
                            nc.scalar.activation(
                                cv[cot][:, rt * 300: (rt + 1) * 300], cps[rt][:],
                                AF.Relu, bias=bn1[:, bi, 1, cot: cot + 1],
                                scale=bn1[:, bi, 0, cot: cot + 1],
                            )
                for i in range(CIC):
                    nc.vector.tensor_add(fsum[i][:], sa_conv[i][:], sc_conv[i][:])

                w6 = sb5.tile([P, 3, CIC, CO], BF16, name="w6")
                b6 = sb5.tile([CO, 3], F32, name="b6", padded_shape=[P, 3])
                for j in range(3):
                    for cic in range(CIC):
                        nc.sync.dma_start(w6[:, j, cic, :], w678g[j, cic])
                for j, bsrc in enumerate((B8, B6, B7)):
                    nc.sync.dma_start(b6[:, j: j + 1], bsrc[:])
                for oi, src in enumerate((fsum, sa_conv, sc_conv)):
                    for hf in range(2):
                        hp = ph5.tile([CO, 450], F32, name="hp", tag="hp",
                                      padded_shape=[P, 450])
                        for cic in range(CIC):
                            nc.tensor.matmul(
                                hp, w6[:, oi, cic, :],
                                src[cic][:, hf * 450: (hf + 1) * 450],
                                start=cic == 0, stop=cic == CIC - 1,
                            )
                        osb = sb5.tile([CO, 450], BF16, name="osb", tag="osb",
                                       padded_shape=[P, 450])
                        nc.vector.tensor_scalar_add(osb[:], hp[:], b6[:, oi: oi + 1])
                        nc.sync.dma_start(OUT[oi, :, hf * 450: (hf + 1) * 450], osb[:])
            ctx.close()

    if split:
        _split_waits(nc)
    return nc


# ------------------------------------------------------------- host side ---

def _bn_fold(p):
    s, b, m, v = np.asarray(p, np.float32)
    a = s / np.sqrt(v + EPS)
    return a.astype(np.float32), (b - m * a).astype(np.float32)


def _bn_layout(a, b):
    # [P, 2, CIC]: [:, 0, c] = a-slice c, [:, 1, c] = b-slice c
    st = np.stack([a.reshape(CIC, P), b.reshape(CIC, P)])   # [2, CIC, P]
    return np.ascontiguousarray(st.transpose(2, 0, 1).astype(np.float32))


def host_prep(inputs):
    """Build the 8 per-core input maps."""
    inp = {k: np.asarray(v) for k, v in inputs.items()}
    x = inp["x"].astype(np.float32)
    d = inp["d"].astype(np.float32)
    lam = np.float32(inp["lamb"])
    B = x.shape[0]

    def w0_blocks(w):
        # [O=512, I=2048, 3, 3] -> per 256-ch slice c: [2, P, 9, CI] lhsT
        out = []
        for c in range(8):
            ws = w[:, c * 256:(c + 1) * 256]          # [512, 256, 3, 3]
            t = np.transpose(ws, (1, 2, 3, 0))        # [I256, 3, 3, O]
            out.append(np.ascontiguousarray(
                t.reshape(2, P, 9, CI).astype(bf)))
        return out

    def conv_w_full(w):
        # [512, 512, 3, 3] -> [9, 4, 128, 512]
        t = np.transpose(w, (2, 3, 1, 0))             # [3,3,512,512]
        return np.ascontiguousarray(t.reshape(9, CIC, P, CI).astype(bf))

    blk_s = w0_blocks(inp["w_s0"])
    blk_c = w0_blocks(inp["w_c0"])

    a0s, b0s = _bn_fold(inp["bn_s0"])
    a0c, b0c = _bn_fold(inp["bn_c0"])
    a1s, b1s = _bn_fold(inp["bn_s1"])
    a1c, b1c = _bn_fold(inp["bn_c1"])
    bn0s = _bn_layout(a0s, b0s)
    bn0c = _bn_layout(a0c, b0c)
    bn1s = _bn_layout(a1s, b1s)
    bn1c = _bn_layout(a1c, b1c)

    wqt = inp["wq"].T.reshape(CIC, P, CQ).astype(np.float32)
    wkt = inp["wk"].T.reshape(CIC, P, CQ).astype(np.float32)
    wqk8 = np.ascontiguousarray(np.stack([wqt, wkt])).reshape(8, QCH)
    wvt8 = np.ascontiguousarray(
        inp["wv"].T.reshape(CIC, P, CI).astype(bf)).reshape(8, VCH)
    w1s8 = conv_w_full(inp["w_s1"]).reshape(8, WCH)
    w1c8 = conv_w_full(inp["w_c1"]).reshape(8, WCH)
    w6t = inp["w6"].T.reshape(CIC, P, CO).astype(bf)
    w7t = inp["w7"].T.reshape(CIC, P, CO).astype(bf)
    w8t = inp["w8"].T.reshape(CIC, P, CO).astype(bf)
    w678_8 = np.ascontiguousarray(np.stack([w8t, w6t, w7t])).reshape(8, OCH)

    gsa = np.full((1, P), np.float32(inp["gamma_sa"]), np.float32)
    gsc = np.full((P, 1), np.float32(inp["gamma_sc"]), np.float32)

    in_maps = []
    for c in range(8):
        b_, s = c // 4, c % 4
        df = d[b_, 0].reshape(NPIX)
        dka = np.zeros((2, MP), np.float32)
        dka[0, :NPIX] = lam * df * df
        dka[0, NPIX:] = -1000.0
        dka[1, :NPIX] = df

        out_r0 = 15 * s
        dqa = np.zeros((2, WINP), np.float32)
        qmask = np.zeros((1, WINP), np.float32)
        dqa[0, :WIN] = 1.0
        for v_ in range(17):
            rv = out_r0 - 1 + v_
            if 0 <= rv < H:
                dqa[1, v_ * H:(v_ + 1) * H] = -2.0 * lam * d[b_, 0, rv]
                qmask[0, v_ * H:(v_ + 1) * H] = 1.0

        in_maps.append({
            "XP": np.ascontiguousarray(
                x[:, c * 256:(c + 1) * 256].reshape(2, 2, P, NPIX).astype(bf)),
            "W0SH": blk_s[c],
            "W0CH": blk_c[c],
            "BN0S": bn0s, "BN0C": bn0c,
            "W1SP": w1s8[c], "W1CP": w1c8[c],
            "WVTP": wvt8[c], "W678P": w678_8[c], "WQKP": wqk8[c],
            "BQ": inp["bq"].reshape(CQ, 1).astype(np.float32),
            "BK": inp["bk"].reshape(CQ, 1).astype(np.float32),
            "BV": np.ascontiguousarray(
                inp["bv"].reshape(CIC, P).T.astype(np.float32)),
            "DKA": dka, "DQA": dqa, "QMASK": qmask,
            "GSA": gsa, "GSC": gsc,
            "BN1S": bn1s, "BN1C": bn1c,
            "B6": inp["b6"].reshape(CO, 1).astype(np.float32),
            "B7": inp["b7"].reshape(CO, 1).astype(np.float32),
            "B8": inp["b8"].reshape(CO, 1).astype(np.float32),
        })
    return in_maps


def assemble(results):
    """results: list of 8 dicts with 'OUT' [3, 40, 900] -> output tuple."""
    outs = []
    for b_ in range(2):
        rows = [np.asarray(results[4 * b_ + s]["OUT"]).astype(np.float32).reshape(
            3, CO, 15, H) for s in range(4)]
        outs.append(np.concatenate(rows, axis=2))        # [3, 40, 60, 60]
    full = np.stack(outs, axis=1)                        # [3, B, 40, 60, 60]
    return full[0], full[1], full[2]


def _split_waits(nc, keep=1):
    """Walrus in this container accepts at most one embedded sync-wait per
    instruction; Tile emits several. Turn extra waits into standalone
    single-wait EventSemaphore instructions before the owner, same engine."""
    n_split = 0
    for fn in nc.m.functions:
        for bb in fn.blocks:
            new_insts = []
            for inst in bb.instructions:
                si = inst.sync_info
                if si is not None and len(si.on_wait) > keep:
                    waits = list(si.on_wait)
                    head, tail = waits[:-keep], waits[-keep:]
                    for j, w in enumerate(head):
                        new_insts.append(mybir.InstEventSemaphore(
                            name=f"{inst.name}-ws{j}",
                            engine=inst.engine,
                            ins=[], outs=[],
                            sync_info=mybir.SyncInfo(on_wait=[w], on_update=[]),
                        ))
                        n_split += 1
                    inst.sync_info = mybir.SyncInfo(
                        on_wait=tail, on_update=list(si.on_update))
                new_insts.append(inst)
            bb.instructions.clear()
            bb.instructions.extend(new_insts)
    return n_split


_NC = None


def kernel(**inputs):
    global _NC
    if _NC is None:
        _NC = build_nc()
    from concourse.bass_utils import run_bass_kernel_spmd
    in_maps = host_prep(inputs)
    res = run_bass_kernel_spmd(_NC, in_maps, core_ids=list(range(8)))
    return assemble(res.results)
